# revision 41
# baseline (speedup 1.0000x reference)
"""Trainium2 Bass kernel for nn_Attention_1580547974448.

Math insight: the reference uses raw .reshape (not a head-split transpose) on
[B,T,H*HD] -> [B,H,T,HD].  With B=4, T=4096, DIM=1024, H=16, HD=64 this makes
each "head" a contiguous 256-row slab of the flattened [B*T, DIM] = [16384,1024]
input: for slab s (rows 256s..256s+255),
    Q = (x_s @ Wq + bq)            viewed row-major as [4096, 64]
    S = Q^T K / sqrt(64)           [64, 64]
    P = softmax(S, axis=-1)
    O = P @ V^T                    [64, 4096], row-major == [256, 1024]
    y_s = O_v @ Wp + bp
i.e. the whole computation is block-diagonal over 64 independent slabs.
We shard 8 slabs (2048 rows) per NeuronCore -> pure data parallel, no
collectives.  Compute dtype bf16 (fp32 PSUM accumulation).

v2 changes vs the 335us baseline (trace-driven):
  - The baseline's GpSimd vvt interleave (5.4us per copy, 173us total) was the
    pipeline serializer: OT waited on it every slab, PE idled ~46us in >1us
    gaps, and each gap re-throttled the PE clock (HAM K=4/8 for 104us; cold
    MMs at 512ns vs warm 259ns).  Fix: evacuate the V^T projection PSUM
    *directly* into the vvt head-interleaved layout, two ScalarE activation
    ops per jt block (src PSUM [64,512] partition-base 64p, dst vvt[:, :, t2]
    free-stride 16).  vt + GpSimd disappear entirely.
  - Q/K and Y matmuls emit jc0/jc1 chains interleaved so consecutive MMs share
    the same stationary operand (weight-swap drain test).
  - ~128 tiny warmup matmuls on the identity at t=0 (consumed via a dummy
    "warm" output so DCE keeps them) pull the HAM un-throttle into the initial
    DMA window.
  - PSUM: qk 2 + v 2 + y 2 + (S/WT/OT shared tag) 2 = 8 banks.

Per-core dataflow (all layouts [partition, free]):
  xt       [128, 8kd, 2048]   x^T, bf16 (host pre-transposed)
  per pair: vvt [64, 512r, 16t2] = head-layout V^T, written straight from the
            8 jt-chain PSUMs by 16 ACT ops (bias fused)
  per slab: q_nat/k_nat [128, 2rt, 1024] (DVE bias evac); S psum [64,64];
            softmax on free dim; WT via PE transpose; O^T 4-chunk PSUMs with
            lhsT = contiguous vvt slices; ovt [128, 8ct, 256]; y = ovt^T @ Wp.
"""

import os
import sys

import numpy as np
import ml_dtypes

import concourse.bass as bass
import concourse.mybir as mybir
import concourse.tile as tile
from concourse import bacc
from concourse.bass_utils import run_bass_kernel_spmd


def _install_ntff_hook_shim():
    """concourse's trace path does `from antenv.axon_hooks import
    get_axon_ntff_profile_hook`; this container's antenv lacks that
    module.  Provide it: a ctypes hook on the axon PJRT .so when
    available (mirrors trn_agent_boot), else a None hook (concourse
    then skips tracing gracefully)."""
    try:
        import antenv.axon_hooks  # noqa: F401
        return
    except ImportError:
        pass
    import contextlib
    import ctypes
    import types

    state = {"hook": None}

    def _build_hook():
        so_path = "/opt/axon/libaxon_pjrt.so"
        if not os.path.exists(so_path):
            return None
        lib = ctypes.CDLL(so_path)
        if not hasattr(lib, "axon_start_nrt_profile"):
            return None
        lib.axon_start_nrt_profile.argtypes = [
            ctypes.POINTER(ctypes.c_int64), ctypes.c_size_t]
        lib.axon_start_nrt_profile.restype = ctypes.c_int64
        lib.axon_stop_nrt_profile.argtypes = [ctypes.c_char_p]
        lib.axon_stop_nrt_profile.restype = ctypes.c_int64

        @contextlib.contextmanager
        def _hook(output_dir, device_ids):
            import jax
            jax.devices()
            if device_ids:
                ids = (ctypes.c_int64 * len(device_ids))(*device_ids)
                rc = lib.axon_start_nrt_profile(ids, len(device_ids))
            else:
                rc = lib.axon_start_nrt_profile(None, 0)
            if rc != 0:
                raise RuntimeError(f"axon_start_nrt_profile rc={rc}")
            try:
                yield
            finally:
                n = lib.axon_stop_nrt_profile(str(output_dir).encode())
                if n < 0:
                    raise RuntimeError(f"axon_stop_nrt_profile rc={n}")
                print(f"profile: {n} file(s) written to {output_dir}")

        return _hook

    def get_axon_ntff_profile_hook():
        if state["hook"] is None:
            try:
                state["hook"] = _build_hook()
            except Exception:
                state["hook"] = None
        return state["hook"]

    mod = types.ModuleType("antenv.axon_hooks")
    mod.get_axon_ntff_profile_hook = get_axon_ntff_profile_hook
    mod.set_axon_ntff_profile_hook = lambda h: state.update(hook=h)
    sys.modules["antenv.axon_hooks"] = mod


_install_ntff_hook_shim()


P = 128          # SBUF partitions
DIM = 1024       # model dim
KD = DIM // P    # 8 contraction tiles
ROWS_PER_CORE = 2048
SLABS_PER_CORE = 8
SLAB = 256       # rows per slab
N_CORES = 8
BF16 = mybir.dt.bfloat16
F32 = mybir.dt.float32

N_WARMUP_MM = 352

_CACHE = {}


def _build_graph():
    nc = bacc.Bacc("TRN2", target_bir_lowering=False, debug=False,
                   num_devices=N_CORES)

    xt_d = nc.dram_tensor("xt", [DIM, ROWS_PER_CORE], BF16, kind="ExternalInput")
    w_d = {
        name: nc.dram_tensor(name, [DIM, DIM], BF16, kind="ExternalInput")
        for name in ("wq", "wk", "wv", "wp")
    }
    bqc_d = nc.dram_tensor("bqc", [P, DIM], F32, kind="ExternalInput")
    bkc_d = nc.dram_tensor("bkc", [P, DIM], F32, kind="ExternalInput")
    bpc_d = nc.dram_tensor("bpc", [P, DIM], F32, kind="ExternalInput")
    bvh_d = nc.dram_tensor("bvh", [64, P], BF16, kind="ExternalInput")
    ident_d = nc.dram_tensor("ident64", [64, 64], BF16, kind="ExternalInput")
    out_d = nc.dram_tensor("out", [ROWS_PER_CORE, DIM], BF16, kind="ExternalOutput")
    warm_d = nc.dram_tensor("warm", [64, 64], F32, kind="ExternalOutput")

    with tile.TileContext(nc) as tc:
        with (
            tc.tile_pool(name="wpool", bufs=1) as wpool,
            tc.tile_pool(name="xpool", bufs=1) as xpool,
            tc.tile_pool(name="bias", bufs=1) as bias_pool,
            tc.tile_pool(name="qk", bufs=2) as qk_pool,
            tc.tile_pool(name="vvt", bufs=2) as vvt_pool,
            tc.tile_pool(name="vt", bufs=1) as vt_pool,
            tc.tile_pool(name="ovt", bufs=2) as ovt_pool,
            tc.tile_pool(name="ysb", bufs=2) as y_pool,
            tc.tile_pool(name="soft", bufs=2) as soft_pool,
            tc.tile_pool(name="ps_qk", bufs=2, space="PSUM") as ps_qk_pool,
            tc.tile_pool(name="ps_v", bufs=2, space="PSUM") as ps_v_pool,
            tc.tile_pool(name="ps_y", bufs=2, space="PSUM") as ps_y_pool,
            tc.tile_pool(name="ps_swo", bufs=2, space="PSUM") as ps_swo_pool,
        ):
            # ---- resident tensors -------------------------------------------
            ident = bias_pool.tile([64, 64], BF16, tag="ident")
            nc.sync.dma_start(ident[:], ident_d[:])

            # PE warmup: dense tiny matmul chain so the HAM un-throttles
            # during the initial weight/activation DMA.  Consumed via the
            # "warm" output so it isn't dead code.
            ps_spam = ps_swo_pool.tile([64, 64], F32, tag="swo")
            for i in range(N_WARMUP_MM):
                nc.tensor.matmul(ps_spam[:], ident[:], ident[:],
                                 start=(i == 0), stop=(i == N_WARMUP_MM - 1))
            warm_sb = bias_pool.tile([64, 64], F32, tag="warm_sb")
            nc.vector.tensor_copy(warm_sb[:, 0:32], ps_spam[:, 0:32])

            bq_bc = bias_pool.tile([P, DIM], F32, tag="bqc")
            bk_bc = bias_pool.tile([P, DIM], F32, tag="bkc")
            bp_bc = bias_pool.tile([P, DIM], F32, tag="bpc")
            bvh_sb = bias_pool.tile([64, P], BF16, tag="bvh")
            nc.sync.dma_start(bq_bc[:], bqc_d[:])
            nc.sync.dma_start(bk_bc[:], bkc_d[:])
            nc.sync.dma_start(bp_bc[:], bpc_d[:])
            nc.sync.dma_start(bvh_sb[:], bvh_d[:])

            # fine-grained loads so pair-0 matmuls can start early
            xt_sb = xpool.tile([P, KD, ROWS_PER_CORE], BF16, tag="xt")
            xt_src = xt_d[:].rearrange("(kd p) r -> p kd r", p=P)
            w_sb = {}
            for name in ("wq", "wk", "wv", "wp"):
                w_sb[name] = wpool.tile([P, KD, DIM], BF16, tag=f"w_{name}",
                                        name=f"w_{name}")
            w_srcs = {name: w_d[name][:].rearrange("(kd p) c -> p kd c", p=P)
                      for name in w_sb}
            # DMA order tuned for startup (V of pair 0 runs first): xt pair-0
            # chunk + wv interleaved, then wq/wk, then bulk wp + xt rest.
            # Later loads use few LARGE calls -- the ~1us SWDGE first-byte
            # per dma_start dominates small transfers.
            nc.sync.dma_start(xt_sb[:, :, 0:512], xt_src[:, :, 0:512])
            for h in range(2):
                # column halves: V chains jt 0-3 start after the first 1MB
                nc.sync.dma_start(w_sb["wv"][:, :, h * 512:(h + 1) * 512],
                                  w_srcs["wv"][:, :, h * 512:(h + 1) * 512])
            for name in ("wq", "wk"):
                for h in range(2):
                    nc.sync.dma_start(w_sb[name][:, h * 4:(h + 1) * 4, :],
                                      w_srcs[name][:, h * 4:(h + 1) * 4, :])
            for h in range(2):
                nc.sync.dma_start(w_sb["wp"][:, h * 4:(h + 1) * 4, :],
                                  w_srcs["wp"][:, h * 4:(h + 1) * 4, :])
            for half in range(1, 4):
                nc.sync.dma_start(
                    xt_sb[:, :, half * 512:(half + 1) * 512],
                    xt_src[:, :, half * 512:(half + 1) * 512])

            # ---- per slab-pair pipeline -------------------------------------
            # Emission (priority) order per pair:
            #   V(pair), part1(even), part1(odd), part2(even), part2(odd)
            # part1 = QK + S + softmax issue, part2 = WT/corr/OT/Y.  The PE
            # transpose in part2(s) sits AFTER the sibling slab's QK+S in the
            # PE stream, so each softmax's ~2.5us DVE/ACT latency hides under
            # the sibling's matmuls -- including for the final pair.

            def emit_v(pair):
                p0 = pair * 2 * SLAB
                # V^T for both slabs, evacuated into the head-interleaved
                # layout vvt[e, r, t2] (t = 16r + t2 flat, so OT's stationary
                # slices are single-free-dim contiguous).  The stride-16
                # scatter costs ~2.4us per [64,512] op on EVERY engine
                # (~5.6ns/elem): one fast contiguous CAST frees the PSUM slot
                # in ~0.7us, then ACT/GpSimd scatter from SBUF with ~10us of
                # deadline slack.  No bias here: the V-bias is folded into
                # the OT evac via the rank-1 correction C = P @ bv_head.
                vvt = vvt_pool.tile([64, 2 * SLAB, 16], BF16, tag="vvt")
                vt_tmp = vt_pool.tile([P, KD, 512], BF16, tag="vt_tmp")
                for jt in range(KD):
                    ps = ps_v_pool.tile([P, 512], F32, tag="ps_v")
                    for kd in range(KD):
                        nc.tensor.matmul(
                            ps[:],
                            w_sb["wv"][:, kd, jt * P:(jt + 1) * P],
                            xt_sb[:, kd, p0: p0 + 512],
                            start=(kd == 0),
                            stop=(kd == KD - 1),
                        )
                    # V^T partition j = 128*jt + 64*par + e -> t2 = 2*jt+par,
                    # dst partition e, free col 16*r + t2.
                    nc.vector.tensor_copy(vt_tmp[:, jt, :], ps[:])
                    nc.scalar.copy(vvt[:, :, 2 * jt], vt_tmp[0:64, jt, :])
                    nc.gpsimd.tensor_copy(vvt[:, :, 2 * jt + 1],
                                          vt_tmp[64:128, jt, :])
                return vvt

            def emit_part1(s, paired):
                c0 = s * SLAB
                # Q, K natural layout (rows on partitions).  paired: jc0/jc1
                # chains interleaved so consecutive MMs share the xt
                # stationary.  Slab 0 runs unpaired so each chain can start
                # as soon as its 512-column weight half has landed.
                q_nat = qk_pool.tile([P, 2, DIM], BF16, tag="q_nat")
                k_nat = qk_pool.tile([P, 2, DIM], BF16, tag="k_nat")
                if not paired:
                    # jc-outer single chains: each starts as soon as its
                    # 512-column weight half has landed (startup slab only).
                    for jc in range(2):
                        for dst_t, wname, bias_bc in (
                            (q_nat, "wq", bq_bc),
                            (k_nat, "wk", bk_bc),
                        ):
                            for rt in range(2):
                                ps_a = ps_qk_pool.tile([P, 512], F32,
                                                       tag="ps_qk")
                                for kd in range(KD):
                                    lhs = xt_sb[:, kd,
                                                c0 + rt * P: c0 + (rt + 1) * P]
                                    nc.tensor.matmul(
                                        ps_a[:], lhs,
                                        w_sb[wname][:, kd,
                                                    jc * 512:(jc + 1) * 512],
                                        start=(kd == 0), stop=(kd == KD - 1))
                                nc.vector.tensor_add(
                                    dst_t[:, rt, jc * 512:(jc + 1) * 512],
                                    ps_a[:],
                                    bias_bc[:, jc * 512:(jc + 1) * 512])
                for rt in range(2 if paired else 0):
                    for dst_t, wname, bias_bc in (
                        (q_nat, "wq", bq_bc),
                        (k_nat, "wk", bk_bc),
                    ):
                        if paired:
                            ps_a = ps_qk_pool.tile([P, 512], F32, tag="ps_qk")
                            ps_b = ps_qk_pool.tile([P, 512], F32, tag="ps_qk")
                            for kd in range(KD):
                                lhs = xt_sb[:, kd,
                                            c0 + rt * P: c0 + (rt + 1) * P]
                                nc.tensor.matmul(
                                    ps_a[:], lhs, w_sb[wname][:, kd, 0:512],
                                    start=(kd == 0), stop=(kd == KD - 1))
                                nc.tensor.matmul(
                                    ps_b[:], lhs, w_sb[wname][:, kd, 512:1024],
                                    start=(kd == 0), stop=(kd == KD - 1))
                            nc.vector.tensor_add(
                                dst_t[:, rt, 0:512], ps_a[:],
                                bias_bc[:, 0:512])
                            nc.vector.tensor_add(
                                dst_t[:, rt, 512:1024], ps_b[:],
                                bias_bc[:, 512:1024])
                        else:
                            pass  # unpaired slabs emit jc-outer below

                # S as 16 [128,128] MMs: each computes a 2x2 block of t2-pair
                # products; only the two diagonal 64x64 blocks are S
                # contributions (off-diagonals are discarded).  Halves the
                # S instruction count vs 32 single-t2 MMs.
                ps_s = ps_swo_pool.tile([P, P], F32, tag="swo")
                n_acc = 0
                for rt in range(2):
                    for tp in range(8):
                        nc.tensor.matmul(
                            ps_s[:],
                            q_nat[:, rt, tp * 128:(tp + 1) * 128],
                            k_nat[:, rt, tp * 128:(tp + 1) * 128],
                            start=(n_acc == 0),
                            stop=(n_acc == 15),
                        )
                        n_acc += 1
                d1_sb = soft_pool.tile([64, 64], F32, tag="d1_sb")
                nc.vector.tensor_copy(d1_sb[:], ps_s[64:128, 64:128])
                s_sb = soft_pool.tile([64, 64], F32, tag="s_sb")
                nc.vector.tensor_add(s_sb[:], ps_s[0:64, 0:64], d1_sb[:])

                # softmax over the free dim (DVE/ACT, overlaps PE)
                negmax = soft_pool.tile([64, 1], F32, tag="negmax")
                nc.vector.reduce_max(negmax[:], s_sb[:],
                                     axis=mybir.AxisListType.X, negate=True)
                p_sb = soft_pool.tile([64, 64], F32, tag="p_sb")
                rsum = soft_pool.tile([64, 1], F32, tag="rsum")
                nc.scalar.activation(p_sb[:], s_sb[:],
                                     mybir.ActivationFunctionType.Exp,
                                     bias=negmax[:], accum_out=rsum[:])
                rinv = soft_pool.tile([64, 1], F32, tag="rinv")
                nc.vector.reciprocal(rinv[:], rsum[:])
                w_soft = soft_pool.tile([64, 64], BF16, tag="w_soft")
                nc.vector.tensor_scalar_mul(w_soft[:], p_sb[:], rinv[:])
                return w_soft

            def emit_part2(s, half, vvt, w_soft, tail=False):
                # WT = W^T via PE transpose
                ps_wt = ps_swo_pool.tile([64, 64], BF16, tag="swo")
                nc.tensor.transpose(ps_wt[:], w_soft[:], ident[:])
                wt_sb = soft_pool.tile([64, 64], BF16, tag="wt_sb")
                nc.vector.tensor_copy(wt_sb[:], ps_wt[:])

                # V-bias correction: corr[16k+t2, d] = sum_e bvh[e,t2] *
                # P[d,e] -- one tiny MM; added to every O^T chunk (whose
                # partition p has t2 = p%16) during the ovt evac.
                corr_ps = ps_swo_pool.tile([P, 64], F32, tag="swo")
                nc.tensor.matmul(corr_ps[:], bvh_sb[:], wt_sb[:],
                                 start=True, stop=True)
                corr4 = soft_pool.tile([P, 4, 64], F32, tag="corr4")
                for t3 in range(4):
                    nc.vector.tensor_copy(corr4[:, t3, :], corr_ps[:])

                # O^T chunks; 4 chunks (t3=0..3) of one ct share a PSUM
                # tile [128, 4, 64], single evac interleaves into ovt.
                # lhsT = contiguous vvt slice (flat head-t columns).
                # In the wind-down (tail) there is no other PE work to cover
                # the pso evac cadence; borrow the idle ps_v banks for a
                # 4-deep rotation.
                ovt = ovt_pool.tile([P, KD, SLAB], BF16, tag="ovt")
                for ct in range(KD):
                    if tail and ct % 2 == 1:
                        pso = ps_v_pool.tile([P, 4, 64], F32, tag="ps_v")
                    else:
                        pso = ps_swo_pool.tile([P, 4, 64], F32, tag="swo")
                    for t3 in range(4):
                        c = 8 * t3 + ct
                        # chunk c: t in [128c, 128c+128) -> r in [8c,8c+8)
                        # contiguous [64, 8, 16] -> opts to [64, 128]
                        lhs = vvt[:, half * SLAB + c * 8:
                                  half * SLAB + (c + 1) * 8, :]
                        nc.tensor.matmul(
                            pso[:, t3, :],
                            lhs,
                            wt_sb[:],
                            start=True, stop=True,
                        )
                    nc.vector.tensor_add(
                        ovt[:, ct, :].rearrange("p (d four) -> p d four",
                                                four=4),
                        pso[:].rearrange("p t3 d -> p d t3"),
                        corr4[:].rearrange("p t3 d -> p d t3"),
                    )

                # Y = OvT^T @ Wp + bp; jc0/jc1 interleaved for stationary
                # reuse -> DMA out
                y_sb = y_pool.tile([P, 2, DIM], BF16, tag="y_sb")
                for rt in range(2):
                    ps_a = ps_y_pool.tile([P, 512], F32, tag="ps_y")
                    ps_b = ps_y_pool.tile([P, 512], F32, tag="ps_y")
                    for ct in range(KD):
                        lhs = ovt[:, ct, rt * P:(rt + 1) * P]
                        nc.tensor.matmul(
                            ps_a[:], lhs, w_sb["wp"][:, ct, 0:512],
                            start=(ct == 0), stop=(ct == KD - 1))
                        nc.tensor.matmul(
                            ps_b[:], lhs, w_sb["wp"][:, ct, 512:1024],
                            start=(ct == 0), stop=(ct == KD - 1))
                    nc.vector.tensor_add(
                        y_sb[:, rt, 0:512], ps_a[:], bp_bc[:, 0:512])
                    nc.vector.tensor_add(
                        y_sb[:, rt, 512:1024], ps_b[:], bp_bc[:, 512:1024])

                out_dst = out_d[s * SLAB:(s + 1) * SLAB, :] \
                    .rearrange("(rt p) c -> p rt c", p=P)
                nc.sync.dma_start(out_dst[:], y_sb[:])

            # V(p+1) is emitted mid-pair so its matmuls are available (and
            # ahead in static PE order) to fill the odd slab's softmax-chain
            # latency -- the sim underestimates that chain under SBUF
            # contention, so give it real filler.
            def emit_tail_spam(n, sl):
                # HAM keep-warm filler during the wind-down: the last pair
                # has no V/QK work left to cover softmax latency, and a few
                # sub-us PE gaps re-throttle the clock to 1.2GHz for the
                # final OT/Y matmuls.  ps_qk is idle by then.  Each chain
                # writes a distinct warm_sb slice so none is dead code.
                ps_f = ps_qk_pool.tile([64, 64], F32, tag="ps_qk")
                for i in range(n):
                    nc.tensor.matmul(ps_f[:], ident[:], ident[:],
                                     start=(i == 0), stop=(i == n - 1))
                nc.vector.tensor_copy(warm_sb[:, sl], ps_f[:, sl])

            vvt_cur = emit_v(0)
            last = SLABS_PER_CORE // 2 - 1
            for pair in range(SLABS_PER_CORE // 2):
                w0 = emit_part1(2 * pair, paired=True)
                w1 = emit_part1(2 * pair + 1, paired=True)
                if pair == last:
                    emit_tail_spam(16, slice(32, 48))
                emit_part2(2 * pair, 0, vvt_cur, w0, tail=(pair == last))
                vvt_next = None
                if pair < last:
                    vvt_next = emit_v(pair + 1)
                else:
                    emit_tail_spam(40, slice(48, 64))
                emit_part2(2 * pair + 1, 1, vvt_cur, w1, tail=(pair == last))
                vvt_cur = vvt_next
            nc.sync.dma_start(warm_d[:], warm_sb[:])

    nc.compile()
    return nc


def _prep_inputs(x, Wq, bq, Wk, bk, Wv, bv, Wp, bp):
    """Host-side shard prep. Returns in_maps list for 8 cores."""
    bf16 = ml_dtypes.bfloat16
    xf = np.ascontiguousarray(np.asarray(x, dtype=np.float32).reshape(-1, DIM))
    scale = np.float32(1.0 / np.sqrt(64.0))

    wq_b = np.ascontiguousarray((np.asarray(Wq) * scale).astype(bf16))
    wk_b = np.ascontiguousarray(np.asarray(Wk).astype(bf16))
    wv_b = np.ascontiguousarray(np.asarray(Wv).astype(bf16))
    wp_b = np.ascontiguousarray(np.asarray(Wp).astype(bf16))

    bqc = np.ascontiguousarray(np.broadcast_to(
        (np.asarray(bq) * scale).astype(np.float32), (P, DIM)))
    bkc = np.ascontiguousarray(np.broadcast_to(
        np.asarray(bk, dtype=np.float32), (P, DIM)))
    bpc = np.ascontiguousarray(np.broadcast_to(
        np.asarray(bp, dtype=np.float32), (P, DIM)))
    # bvh[e, 16k + t2] = bv[64*t2 + e]: lhsT of the per-slab V-bias
    # correction MM (out partition p=16k+t2 gets C[d, t2]).
    bvf = np.asarray(bv, dtype=np.float32).reshape(16, 64)  # [t2, e]
    bvh = np.ascontiguousarray(
        np.tile(bvf.T, (1, 8)).astype(bf16))                # [64, 128]
    ident = np.eye(64, dtype=bf16)

    shared = {
        "wq": wq_b, "wk": wk_b, "wv": wv_b, "wp": wp_b,
        "bqc": bqc, "bkc": bkc, "bpc": bpc, "bvh": bvh,
        "ident64": ident,
    }
    in_maps = []
    for c in range(N_CORES):
        xs = xf[c * ROWS_PER_CORE:(c + 1) * ROWS_PER_CORE]  # [2048, 1024]
        xt = np.ascontiguousarray(xs.T.astype(bf16))        # [1024, 2048]
        in_maps.append({"xt": xt, **shared})
    return in_maps


def kernel(x, Wq, bq, Wk, bk, Wv, bv, Wp, bp):
    if "nc" not in _CACHE:
        _CACHE["nc"] = _build_graph()
    nc = _CACHE["nc"]

    in_maps = _prep_inputs(x, Wq, bq, Wk, bk, Wv, bv, Wp, bp)
    trace = bool(int(os.environ.get("ATHENA_TRACE", "0")))
    res = run_bass_kernel_spmd(nc, in_maps, core_ids=list(range(N_CORES)),
                               trace=trace)
    _CACHE["last_exec_time_ns"] = res.exec_time_ns

    out = np.concatenate([res.results[c]["out"] for c in range(N_CORES)], axis=0)
    return np.ascontiguousarray(out.reshape(np.asarray(x).shape)
                                .astype(np.float32))


# revision 42
# speedup vs baseline: 1.2028x; 1.2028x over previous
"""Trainium2 Bass kernel for nn_Attention_1580547974448.

Math insight: the reference uses raw .reshape (not a head-split transpose) on
[B,T,H*HD] -> [B,H,T,HD].  With B=4, T=4096, DIM=1024, H=16, HD=64 this makes
each "head" a contiguous 256-row slab of the flattened [B*T, DIM] = [16384,1024]
input: for slab s (rows 256s..256s+255),
    Q = (x_s @ Wq + bq)            viewed row-major as [4096, 64]
    S = Q^T K / sqrt(64)           [64, 64]
    P = softmax(S, axis=-1)
    O = P @ V^T                    [64, 4096], row-major == [256, 1024]
    y_s = O_v @ Wp + bp
i.e. the whole computation is block-diagonal over 64 independent slabs.
We shard 8 slabs (2048 rows) per NeuronCore -> pure data parallel, no
collectives.  Compute dtype bf16 (fp32 PSUM accumulation).

Optimizations vs the 335us baseline (trace-driven; best ~301us cool,
~360us when the chip is P0 power-throttled to 2.0GHz):
  - Baseline's serializer was a GpSimd head-interleave of V^T (5.4us/copy,
    173us total): OT waited on it every slab, PE idled 46us and HAM
    re-throttled the clock (cold MMs 512ns vs warm 216ns).  The t2-to-free
    interleave is a stride-16 2B scatter costing ~2.4us per [64,512] op on
    EVERY engine (~5.6ns/elem, SBUF write RMW); it cannot be avoided (the
    matmul stationary AP must be single-free-dim) but it CAN be pipelined:
    one fast contiguous DVE CAST frees the V PSUM slot in ~0.7us, then ACT
    (even t2) and GpSimd (odd t2) scatter from SBUF with ~10us of slack.
  - V-bias folded into the OT evac as a rank-1 correction C = P @ bv_head
    (one [64x128x64] MM off WT per slab, added during the ovt evac), so
    scatters are plain copies any engine can run.
  - Q/K and Y emit jc0/jc1 chains interleaved: consecutive MMs share the
    stationary operand; a same-stationary MM issues at the 216ns stream
    floor vs ~270ns with a weight swap.
  - Emission order = scheduler priority: per pair, part1 = QK+S+softmax
    issue for BOTH slabs, then part2 = WT/corr/OT/Y, with V(p+1) emitted
    mid-pair -- each softmax's ~2.5us latency hides under sibling matmuls.
  - S computed as 16 [128,contract]x[128] MMs (t2-pairs; diagonal 64x64
    blocks summed by one DVE add) instead of 32 single-t2 MMs.
  - HAM keep-warm: ~352 tiny identity matmuls cover the DMA-bound start
    (DMA queues only begin ~8.6us in; ~195GB/s effective), small spam
    chains + ps_v-borrowed OT PSUM rotation keep the clock warm through
    the wind-down.  All spam consumed via the "warm" output (anti-DCE).
  - DMA: few large calls (1us SWDGE first-byte per call), wv split in
    column halves so V chains start on the first 1MB; bf16 output.
  - PSUM: qk 2 + v 2 + y 2 + (S/WT/corr/OT shared tag) 2 = 8 banks.

Per-core dataflow (all layouts [partition, free]):
  xt       [128, 8kd, 2048]   x^T, bf16 (host pre-transposed)
  per pair: vvt [64, 512r, 16t2] = head-layout V^T (flat col = head t)
  per slab: q_nat/k_nat [128, 2rt, 1024] (DVE bias evac); S psum [128,128];
            softmax on free dim; WT via PE transpose; O^T 4-chunk PSUMs with
            lhsT = contiguous vvt slices; ovt [128, 8ct, 256]; y = ovt^T @ Wp.
"""

import os
import sys

import numpy as np
import ml_dtypes

import concourse.bass as bass
import concourse.mybir as mybir
import concourse.tile as tile
from concourse import bacc
from concourse.bass_utils import run_bass_kernel_spmd


def _install_ntff_hook_shim():
    """concourse's trace path does `from antenv.axon_hooks import
    get_axon_ntff_profile_hook`; this container's antenv lacks that
    module.  Provide it: a ctypes hook on the axon PJRT .so when
    available (mirrors trn_agent_boot), else a None hook (concourse
    then skips tracing gracefully)."""
    try:
        import antenv.axon_hooks  # noqa: F401
        return
    except ImportError:
        pass
    import contextlib
    import ctypes
    import types

    state = {"hook": None}

    def _build_hook():
        so_path = "/opt/axon/libaxon_pjrt.so"
        if not os.path.exists(so_path):
            return None
        lib = ctypes.CDLL(so_path)
        if not hasattr(lib, "axon_start_nrt_profile"):
            return None
        lib.axon_start_nrt_profile.argtypes = [
            ctypes.POINTER(ctypes.c_int64), ctypes.c_size_t]
        lib.axon_start_nrt_profile.restype = ctypes.c_int64
        lib.axon_stop_nrt_profile.argtypes = [ctypes.c_char_p]
        lib.axon_stop_nrt_profile.restype = ctypes.c_int64

        @contextlib.contextmanager
        def _hook(output_dir, device_ids):
            import jax
            jax.devices()
            if device_ids:
                ids = (ctypes.c_int64 * len(device_ids))(*device_ids)
                rc = lib.axon_start_nrt_profile(ids, len(device_ids))
            else:
                rc = lib.axon_start_nrt_profile(None, 0)
            if rc != 0:
                raise RuntimeError(f"axon_start_nrt_profile rc={rc}")
            try:
                yield
            finally:
                n = lib.axon_stop_nrt_profile(str(output_dir).encode())
                if n < 0:
                    raise RuntimeError(f"axon_stop_nrt_profile rc={n}")
                print(f"profile: {n} file(s) written to {output_dir}")

        return _hook

    def get_axon_ntff_profile_hook():
        if state["hook"] is None:
            try:
                state["hook"] = _build_hook()
            except Exception:
                state["hook"] = None
        return state["hook"]

    mod = types.ModuleType("antenv.axon_hooks")
    mod.get_axon_ntff_profile_hook = get_axon_ntff_profile_hook
    mod.set_axon_ntff_profile_hook = lambda h: state.update(hook=h)
    sys.modules["antenv.axon_hooks"] = mod


_install_ntff_hook_shim()


P = 128          # SBUF partitions
DIM = 1024       # model dim
KD = DIM // P    # 8 contraction tiles
ROWS_PER_CORE = 2048
SLABS_PER_CORE = 8
SLAB = 256       # rows per slab
N_CORES = 8
BF16 = mybir.dt.bfloat16
F32 = mybir.dt.float32

N_WARMUP_MM = 352

_CACHE = {}


def _build_graph():
    nc = bacc.Bacc("TRN2", target_bir_lowering=False, debug=False,
                   num_devices=N_CORES)

    xt_d = nc.dram_tensor("xt", [DIM, ROWS_PER_CORE], BF16, kind="ExternalInput")
    w_d = {
        name: nc.dram_tensor(name, [DIM, DIM], BF16, kind="ExternalInput")
        for name in ("wq", "wk", "wv", "wp")
    }
    bqc_d = nc.dram_tensor("bqc", [P, DIM], F32, kind="ExternalInput")
    bkc_d = nc.dram_tensor("bkc", [P, DIM], F32, kind="ExternalInput")
    bpc_d = nc.dram_tensor("bpc", [P, DIM], F32, kind="ExternalInput")
    bvh_d = nc.dram_tensor("bvh", [64, P], BF16, kind="ExternalInput")
    ident_d = nc.dram_tensor("ident64", [64, 64], BF16, kind="ExternalInput")
    out_d = nc.dram_tensor("out", [ROWS_PER_CORE, DIM], BF16, kind="ExternalOutput")
    warm_d = nc.dram_tensor("warm", [64, 64], F32, kind="ExternalOutput")

    with tile.TileContext(nc) as tc:
        with (
            tc.tile_pool(name="wpool", bufs=1) as wpool,
            tc.tile_pool(name="xpool", bufs=1) as xpool,
            tc.tile_pool(name="bias", bufs=1) as bias_pool,
            tc.tile_pool(name="qk", bufs=2) as qk_pool,
            tc.tile_pool(name="vvt", bufs=2) as vvt_pool,
            tc.tile_pool(name="vt", bufs=1) as vt_pool,
            tc.tile_pool(name="ovt", bufs=2) as ovt_pool,
            tc.tile_pool(name="ysb", bufs=2) as y_pool,
            tc.tile_pool(name="soft", bufs=2) as soft_pool,
            tc.tile_pool(name="ps_qk", bufs=2, space="PSUM") as ps_qk_pool,
            tc.tile_pool(name="ps_v", bufs=2, space="PSUM") as ps_v_pool,
            tc.tile_pool(name="ps_y", bufs=2, space="PSUM") as ps_y_pool,
            tc.tile_pool(name="ps_swo", bufs=2, space="PSUM") as ps_swo_pool,
        ):
            # ---- resident tensors -------------------------------------------
            ident = bias_pool.tile([64, 64], BF16, tag="ident")
            nc.sync.dma_start(ident[:], ident_d[:])

            # PE warmup: dense tiny matmul chain so the HAM un-throttles
            # during the initial weight/activation DMA.  Consumed via the
            # "warm" output so it isn't dead code.
            ps_spam = ps_swo_pool.tile([64, 64], F32, tag="swo")
            for i in range(N_WARMUP_MM):
                nc.tensor.matmul(ps_spam[:], ident[:], ident[:],
                                 start=(i == 0), stop=(i == N_WARMUP_MM - 1))
            warm_sb = bias_pool.tile([64, 64], F32, tag="warm_sb")
            nc.vector.tensor_copy(warm_sb[:, 0:32], ps_spam[:, 0:32])

            bq_bc = bias_pool.tile([P, DIM], F32, tag="bqc")
            bk_bc = bias_pool.tile([P, DIM], F32, tag="bkc")
            bp_bc = bias_pool.tile([P, DIM], F32, tag="bpc")
            bvh_sb = bias_pool.tile([64, P], BF16, tag="bvh")
            nc.sync.dma_start(bq_bc[:], bqc_d[:])
            nc.sync.dma_start(bk_bc[:], bkc_d[:])
            nc.sync.dma_start(bp_bc[:], bpc_d[:])
            nc.sync.dma_start(bvh_sb[:], bvh_d[:])

            # fine-grained loads so pair-0 matmuls can start early
            xt_sb = xpool.tile([P, KD, ROWS_PER_CORE], BF16, tag="xt")
            xt_src = xt_d[:].rearrange("(kd p) r -> p kd r", p=P)
            w_sb = {}
            for name in ("wq", "wk", "wv", "wp"):
                w_sb[name] = wpool.tile([P, KD, DIM], BF16, tag=f"w_{name}",
                                        name=f"w_{name}")
            w_srcs = {name: w_d[name][:].rearrange("(kd p) c -> p kd c", p=P)
                      for name in w_sb}
            # DMA order tuned for startup (V of pair 0 runs first): xt pair-0
            # chunk + wv interleaved, then wq/wk, then bulk wp + xt rest.
            # Later loads use few LARGE calls -- the ~1us SWDGE first-byte
            # per dma_start dominates small transfers.
            nc.sync.dma_start(xt_sb[:, :, 0:512], xt_src[:, :, 0:512])
            for h in range(2):
                # column halves: V chains jt 0-3 start after the first 1MB
                nc.sync.dma_start(w_sb["wv"][:, :, h * 512:(h + 1) * 512],
                                  w_srcs["wv"][:, :, h * 512:(h + 1) * 512])
            for name in ("wq", "wk"):
                for h in range(2):
                    nc.sync.dma_start(w_sb[name][:, h * 4:(h + 1) * 4, :],
                                      w_srcs[name][:, h * 4:(h + 1) * 4, :])
            for h in range(2):
                nc.sync.dma_start(w_sb["wp"][:, h * 4:(h + 1) * 4, :],
                                  w_srcs["wp"][:, h * 4:(h + 1) * 4, :])
            for half in range(1, 4):
                nc.sync.dma_start(
                    xt_sb[:, :, half * 512:(half + 1) * 512],
                    xt_src[:, :, half * 512:(half + 1) * 512])

            # ---- per slab-pair pipeline -------------------------------------
            # Emission (priority) order per pair:
            #   V(pair), part1(even), part1(odd), part2(even), part2(odd)
            # part1 = QK + S + softmax issue, part2 = WT/corr/OT/Y.  The PE
            # transpose in part2(s) sits AFTER the sibling slab's QK+S in the
            # PE stream, so each softmax's ~2.5us DVE/ACT latency hides under
            # the sibling's matmuls -- including for the final pair.

            def emit_v(pair):
                p0 = pair * 2 * SLAB
                # V^T for both slabs, evacuated into the head-interleaved
                # layout vvt[e, r, t2] (t = 16r + t2 flat, so OT's stationary
                # slices are single-free-dim contiguous).  The stride-16
                # scatter costs ~2.4us per [64,512] op on EVERY engine
                # (~5.6ns/elem): one fast contiguous CAST frees the PSUM slot
                # in ~0.7us, then ACT/GpSimd scatter from SBUF with ~10us of
                # deadline slack.  No bias here: the V-bias is folded into
                # the OT evac via the rank-1 correction C = P @ bv_head.
                vvt = vvt_pool.tile([64, 2 * SLAB, 16], BF16, tag="vvt")
                vt_tmp = vt_pool.tile([P, KD, 512], BF16, tag="vt_tmp")
                for jt in range(KD):
                    ps = ps_v_pool.tile([P, 512], F32, tag="ps_v")
                    for kd in range(KD):
                        nc.tensor.matmul(
                            ps[:],
                            w_sb["wv"][:, kd, jt * P:(jt + 1) * P],
                            xt_sb[:, kd, p0: p0 + 512],
                            start=(kd == 0),
                            stop=(kd == KD - 1),
                        )
                    # V^T partition j = 128*jt + 64*par + e -> t2 = 2*jt+par,
                    # dst partition e, free col 16*r + t2.
                    nc.vector.tensor_copy(vt_tmp[:, jt, :], ps[:])
                    nc.scalar.copy(vvt[:, :, 2 * jt], vt_tmp[0:64, jt, :])
                    nc.gpsimd.tensor_copy(vvt[:, :, 2 * jt + 1],
                                          vt_tmp[64:128, jt, :])
                return vvt

            def emit_part1(s, paired):
                c0 = s * SLAB
                # Q, K natural layout (rows on partitions).  paired: jc0/jc1
                # chains interleaved so consecutive MMs share the xt
                # stationary.  Slab 0 runs unpaired so each chain can start
                # as soon as its 512-column weight half has landed.
                q_nat = qk_pool.tile([P, 2, DIM], BF16, tag="q_nat")
                k_nat = qk_pool.tile([P, 2, DIM], BF16, tag="k_nat")
                if not paired:
                    # jc-outer single chains: each starts as soon as its
                    # 512-column weight half has landed (startup slab only).
                    for jc in range(2):
                        for dst_t, wname, bias_bc in (
                            (q_nat, "wq", bq_bc),
                            (k_nat, "wk", bk_bc),
                        ):
                            for rt in range(2):
                                ps_a = ps_qk_pool.tile([P, 512], F32,
                                                       tag="ps_qk")
                                for kd in range(KD):
                                    lhs = xt_sb[:, kd,
                                                c0 + rt * P: c0 + (rt + 1) * P]
                                    nc.tensor.matmul(
                                        ps_a[:], lhs,
                                        w_sb[wname][:, kd,
                                                    jc * 512:(jc + 1) * 512],
                                        start=(kd == 0), stop=(kd == KD - 1))
                                nc.vector.tensor_add(
                                    dst_t[:, rt, jc * 512:(jc + 1) * 512],
                                    ps_a[:],
                                    bias_bc[:, jc * 512:(jc + 1) * 512])
                for rt in range(2 if paired else 0):
                    for dst_t, wname, bias_bc in (
                        (q_nat, "wq", bq_bc),
                        (k_nat, "wk", bk_bc),
                    ):
                        if paired:
                            ps_a = ps_qk_pool.tile([P, 512], F32, tag="ps_qk")
                            ps_b = ps_qk_pool.tile([P, 512], F32, tag="ps_qk")
                            for kd in range(KD):
                                lhs = xt_sb[:, kd,
                                            c0 + rt * P: c0 + (rt + 1) * P]
                                nc.tensor.matmul(
                                    ps_a[:], lhs, w_sb[wname][:, kd, 0:512],
                                    start=(kd == 0), stop=(kd == KD - 1))
                                nc.tensor.matmul(
                                    ps_b[:], lhs, w_sb[wname][:, kd, 512:1024],
                                    start=(kd == 0), stop=(kd == KD - 1))
                            nc.vector.tensor_add(
                                dst_t[:, rt, 0:512], ps_a[:],
                                bias_bc[:, 0:512])
                            nc.vector.tensor_add(
                                dst_t[:, rt, 512:1024], ps_b[:],
                                bias_bc[:, 512:1024])
                        else:
                            pass  # unpaired slabs emit jc-outer below

                # S as 16 [128,128] MMs: each computes a 2x2 block of t2-pair
                # products; only the two diagonal 64x64 blocks are S
                # contributions (off-diagonals are discarded).  Halves the
                # S instruction count vs 32 single-t2 MMs.
                ps_s = ps_swo_pool.tile([P, P], F32, tag="swo")
                n_acc = 0
                for rt in range(2):
                    for tp in range(8):
                        nc.tensor.matmul(
                            ps_s[:],
                            q_nat[:, rt, tp * 128:(tp + 1) * 128],
                            k_nat[:, rt, tp * 128:(tp + 1) * 128],
                            start=(n_acc == 0),
                            stop=(n_acc == 15),
                        )
                        n_acc += 1
                d1_sb = soft_pool.tile([64, 64], F32, tag="d1_sb")
                nc.vector.tensor_copy(d1_sb[:], ps_s[64:128, 64:128])
                s_sb = soft_pool.tile([64, 64], F32, tag="s_sb")
                nc.vector.tensor_add(s_sb[:], ps_s[0:64, 0:64], d1_sb[:])

                # softmax over the free dim (DVE/ACT, overlaps PE)
                negmax = soft_pool.tile([64, 1], F32, tag="negmax")
                nc.vector.reduce_max(negmax[:], s_sb[:],
                                     axis=mybir.AxisListType.X, negate=True)
                p_sb = soft_pool.tile([64, 64], F32, tag="p_sb")
                rsum = soft_pool.tile([64, 1], F32, tag="rsum")
                nc.scalar.activation(p_sb[:], s_sb[:],
                                     mybir.ActivationFunctionType.Exp,
                                     bias=negmax[:], accum_out=rsum[:])
                rinv = soft_pool.tile([64, 1], F32, tag="rinv")
                nc.vector.reciprocal(rinv[:], rsum[:])
                w_soft = soft_pool.tile([64, 64], BF16, tag="w_soft")
                nc.vector.tensor_scalar_mul(w_soft[:], p_sb[:], rinv[:])
                return w_soft

            def emit_part2(s, half, vvt, w_soft, tail=False):
                # WT = W^T via PE transpose
                ps_wt = ps_swo_pool.tile([64, 64], BF16, tag="swo")
                nc.tensor.transpose(ps_wt[:], w_soft[:], ident[:])
                wt_sb = soft_pool.tile([64, 64], BF16, tag="wt_sb")
                nc.vector.tensor_copy(wt_sb[:], ps_wt[:])

                # V-bias correction: corr[16k+t2, d] = sum_e bvh[e,t2] *
                # P[d,e] -- one tiny MM; added to every O^T chunk (whose
                # partition p has t2 = p%16) during the ovt evac.
                corr_ps = ps_swo_pool.tile([P, 64], F32, tag="swo")
                nc.tensor.matmul(corr_ps[:], bvh_sb[:], wt_sb[:],
                                 start=True, stop=True)
                corr4 = soft_pool.tile([P, 4, 64], F32, tag="corr4")
                for t3 in range(4):
                    nc.vector.tensor_copy(corr4[:, t3, :], corr_ps[:])

                # O^T chunks; 4 chunks (t3=0..3) of one ct share a PSUM
                # tile [128, 4, 64], single evac interleaves into ovt.
                # lhsT = contiguous vvt slice (flat head-t columns).
                # In the wind-down (tail) there is no other PE work to cover
                # the pso evac cadence; borrow the idle ps_v banks for a
                # 4-deep rotation.
                ovt = ovt_pool.tile([P, KD, SLAB], BF16, tag="ovt")
                for ct in range(KD):
                    if tail and ct % 2 == 1:
                        pso = ps_v_pool.tile([P, 4, 64], F32, tag="ps_v")
                    else:
                        pso = ps_swo_pool.tile([P, 4, 64], F32, tag="swo")
                    for t3 in range(4):
                        c = 8 * t3 + ct
                        # chunk c: t in [128c, 128c+128) -> r in [8c,8c+8)
                        # contiguous [64, 8, 16] -> opts to [64, 128]
                        lhs = vvt[:, half * SLAB + c * 8:
                                  half * SLAB + (c + 1) * 8, :]
                        nc.tensor.matmul(
                            pso[:, t3, :],
                            lhs,
                            wt_sb[:],
                            start=True, stop=True,
                        )
                    nc.vector.tensor_add(
                        ovt[:, ct, :].rearrange("p (d four) -> p d four",
                                                four=4),
                        pso[:].rearrange("p t3 d -> p d t3"),
                        corr4[:].rearrange("p t3 d -> p d t3"),
                    )

                # Y = OvT^T @ Wp + bp; jc0/jc1 interleaved for stationary
                # reuse -> DMA out
                y_sb = y_pool.tile([P, 2, DIM], BF16, tag="y_sb")
                for rt in range(2):
                    ps_a = ps_y_pool.tile([P, 512], F32, tag="ps_y")
                    ps_b = ps_y_pool.tile([P, 512], F32, tag="ps_y")
                    for ct in range(KD):
                        lhs = ovt[:, ct, rt * P:(rt + 1) * P]
                        nc.tensor.matmul(
                            ps_a[:], lhs, w_sb["wp"][:, ct, 0:512],
                            start=(ct == 0), stop=(ct == KD - 1))
                        nc.tensor.matmul(
                            ps_b[:], lhs, w_sb["wp"][:, ct, 512:1024],
                            start=(ct == 0), stop=(ct == KD - 1))
                    nc.vector.tensor_add(
                        y_sb[:, rt, 0:512], ps_a[:], bp_bc[:, 0:512])
                    nc.vector.tensor_add(
                        y_sb[:, rt, 512:1024], ps_b[:], bp_bc[:, 512:1024])

                out_dst = out_d[s * SLAB:(s + 1) * SLAB, :] \
                    .rearrange("(rt p) c -> p rt c", p=P)
                nc.sync.dma_start(out_dst[:], y_sb[:])

            # V(p+1) is emitted mid-pair so its matmuls are available (and
            # ahead in static PE order) to fill the odd slab's softmax-chain
            # latency -- the sim underestimates that chain under SBUF
            # contention, so give it real filler.
            def emit_tail_spam(n, sl):
                # HAM keep-warm filler during the wind-down: the last pair
                # has no V/QK work left to cover softmax latency, and a few
                # sub-us PE gaps re-throttle the clock to 1.2GHz for the
                # final OT/Y matmuls.  ps_qk is idle by then.  Each chain
                # writes a distinct warm_sb slice so none is dead code.
                ps_f = ps_qk_pool.tile([64, 64], F32, tag="ps_qk")
                for i in range(n):
                    nc.tensor.matmul(ps_f[:], ident[:], ident[:],
                                     start=(i == 0), stop=(i == n - 1))
                nc.vector.tensor_copy(warm_sb[:, sl], ps_f[:, sl])

            vvt_cur = emit_v(0)
            last = SLABS_PER_CORE // 2 - 1
            for pair in range(SLABS_PER_CORE // 2):
                w0 = emit_part1(2 * pair, paired=True)
                w1 = emit_part1(2 * pair + 1, paired=True)
                if pair == last:
                    emit_tail_spam(16, slice(32, 48))
                emit_part2(2 * pair, 0, vvt_cur, w0, tail=(pair == last))
                vvt_next = None
                if pair < last:
                    vvt_next = emit_v(pair + 1)
                else:
                    emit_tail_spam(40, slice(48, 64))
                emit_part2(2 * pair + 1, 1, vvt_cur, w1, tail=(pair == last))
                vvt_cur = vvt_next
            nc.sync.dma_start(warm_d[:], warm_sb[:])

    nc.compile()
    return nc


def _prep_inputs(x, Wq, bq, Wk, bk, Wv, bv, Wp, bp):
    """Host-side shard prep. Returns in_maps list for 8 cores."""
    bf16 = ml_dtypes.bfloat16
    xf = np.ascontiguousarray(np.asarray(x, dtype=np.float32).reshape(-1, DIM))
    scale = np.float32(1.0 / np.sqrt(64.0))

    wq_b = np.ascontiguousarray((np.asarray(Wq) * scale).astype(bf16))
    wk_b = np.ascontiguousarray(np.asarray(Wk).astype(bf16))
    wv_b = np.ascontiguousarray(np.asarray(Wv).astype(bf16))
    wp_b = np.ascontiguousarray(np.asarray(Wp).astype(bf16))

    bqc = np.ascontiguousarray(np.broadcast_to(
        (np.asarray(bq) * scale).astype(np.float32), (P, DIM)))
    bkc = np.ascontiguousarray(np.broadcast_to(
        np.asarray(bk, dtype=np.float32), (P, DIM)))
    bpc = np.ascontiguousarray(np.broadcast_to(
        np.asarray(bp, dtype=np.float32), (P, DIM)))
    # bvh[e, 16k + t2] = bv[64*t2 + e]: lhsT of the per-slab V-bias
    # correction MM (out partition p=16k+t2 gets C[d, t2]).
    bvf = np.asarray(bv, dtype=np.float32).reshape(16, 64)  # [t2, e]
    bvh = np.ascontiguousarray(
        np.tile(bvf.T, (1, 8)).astype(bf16))                # [64, 128]
    ident = np.eye(64, dtype=bf16)

    shared = {
        "wq": wq_b, "wk": wk_b, "wv": wv_b, "wp": wp_b,
        "bqc": bqc, "bkc": bkc, "bpc": bpc, "bvh": bvh,
        "ident64": ident,
    }
    in_maps = []
    for c in range(N_CORES):
        xs = xf[c * ROWS_PER_CORE:(c + 1) * ROWS_PER_CORE]  # [2048, 1024]
        xt = np.ascontiguousarray(xs.T.astype(bf16))        # [1024, 2048]
        in_maps.append({"xt": xt, **shared})
    return in_maps


def kernel(x, Wq, bq, Wk, bk, Wv, bv, Wp, bp):
    if "nc" not in _CACHE:
        _CACHE["nc"] = _build_graph()
    nc = _CACHE["nc"]

    in_maps = _prep_inputs(x, Wq, bq, Wk, bk, Wv, bv, Wp, bp)
    trace = bool(int(os.environ.get("ATHENA_TRACE", "0")))
    res = run_bass_kernel_spmd(nc, in_maps, core_ids=list(range(N_CORES)),
                               trace=trace)
    _CACHE["last_exec_time_ns"] = res.exec_time_ns

    out = np.concatenate([res.results[c]["out"] for c in range(N_CORES)], axis=0)
    return np.ascontiguousarray(out.reshape(np.asarray(x).shape)
                                .astype(np.float32))


# revision 49
# speedup vs baseline: 1.2130x; 1.0085x over previous
"""Trainium2 Bass kernel for nn_Attention_1580547974448.

Math insight: the reference uses raw .reshape (not a head-split transpose) on
[B,T,H*HD] -> [B,H,T,HD].  With B=4, T=4096, DIM=1024, H=16, HD=64 this makes
each "head" a contiguous 256-row slab of the flattened [B*T, DIM] = [16384,1024]
input: for slab s (rows 256s..256s+255),
    Q = (x_s @ Wq + bq)            viewed row-major as [4096, 64]
    S = Q^T K / sqrt(64)           [64, 64]
    P = softmax(S, axis=-1)
    O = P @ V^T                    [64, 4096], row-major == [256, 1024]
    y_s = O_v @ Wp + bp
i.e. the whole computation is block-diagonal over 64 independent slabs.
We shard 8 slabs (2048 rows) per NeuronCore -> pure data parallel, no
collectives.  Compute dtype bf16 (fp32 PSUM accumulation).

Optimizations vs the 335us baseline (trace-driven; best ~301us cool,
~360us when the chip is P0 power-throttled to 2.0GHz):
  - Baseline's serializer was a GpSimd head-interleave of V^T (5.4us/copy,
    173us total): OT waited on it every slab, PE idled 46us and HAM
    re-throttled the clock (cold MMs 512ns vs warm 216ns).  The t2-to-free
    interleave is a stride-16 2B scatter costing ~2.4us per [64,512] op on
    EVERY engine (~5.6ns/elem, SBUF write RMW); it cannot be avoided (the
    matmul stationary AP must be single-free-dim) but it CAN be pipelined:
    one fast contiguous DVE CAST frees the V PSUM slot in ~0.7us, then ACT
    (even t2) and GpSimd (odd t2) scatter from SBUF with ~10us of slack.
  - V-bias folded into the OT evac as a rank-1 correction C = P @ bv_head
    (one [64x128x64] MM off WT per slab, added during the ovt evac), so
    scatters are plain copies any engine can run.
  - Q/K and Y emit jc0/jc1 chains interleaved: consecutive MMs share the
    stationary operand; a same-stationary MM issues at the 216ns stream
    floor vs ~270ns with a weight swap.
  - Emission order = scheduler priority: per pair, part1 = QK+S+softmax
    issue for BOTH slabs, then part2 = WT/corr/OT/Y, with V(p+1) emitted
    mid-pair -- each softmax's ~2.5us latency hides under sibling matmuls.
  - S computed as 16 [128,contract]x[128] MMs (t2-pairs; diagonal 64x64
    blocks summed by one DVE add) instead of 32 single-t2 MMs.
  - HAM keep-warm: ~352 tiny identity matmuls cover the DMA-bound start
    (DMA queues only begin ~8.6us in; ~195GB/s effective), small spam
    chains + ps_v-borrowed OT PSUM rotation keep the clock warm through
    the wind-down.  All spam consumed via the "warm" output (anti-DCE).
  - DMA: few large calls (1us SWDGE first-byte per call), wv split in
    column halves so V chains start on the first 1MB; bf16 output.
  - PSUM: qk 2 + v 2 + y 2 + (S/WT/corr/OT shared tag) 2 = 8 banks.

Per-core dataflow (all layouts [partition, free]):
  xt       [128, 8kd, 2048]   x^T, bf16 (host pre-transposed)
  per pair: vvt [64, 512r, 16t2] = head-layout V^T (flat col = head t)
  per slab: q_nat/k_nat [128, 2rt, 1024] (DVE bias evac); S psum [128,128];
            softmax on free dim; WT via PE transpose; O^T 4-chunk PSUMs with
            lhsT = contiguous vvt slices; ovt [128, 8ct, 256]; y = ovt^T @ Wp.
"""

import os
import sys

import numpy as np
import ml_dtypes

import concourse.bass as bass
import concourse.mybir as mybir
import concourse.tile as tile
from concourse import bacc
from concourse.bass_utils import run_bass_kernel_spmd


def _install_ntff_hook_shim():
    """concourse's trace path does `from antenv.axon_hooks import
    get_axon_ntff_profile_hook`; this container's antenv lacks that
    module.  Provide it: a ctypes hook on the axon PJRT .so when
    available (mirrors trn_agent_boot), else a None hook (concourse
    then skips tracing gracefully)."""
    try:
        import antenv.axon_hooks  # noqa: F401
        return
    except ImportError:
        pass
    import contextlib
    import ctypes
    import types

    state = {"hook": None}

    def _build_hook():
        so_path = "/opt/axon/libaxon_pjrt.so"
        if not os.path.exists(so_path):
            return None
        lib = ctypes.CDLL(so_path)
        if not hasattr(lib, "axon_start_nrt_profile"):
            return None
        lib.axon_start_nrt_profile.argtypes = [
            ctypes.POINTER(ctypes.c_int64), ctypes.c_size_t]
        lib.axon_start_nrt_profile.restype = ctypes.c_int64
        lib.axon_stop_nrt_profile.argtypes = [ctypes.c_char_p]
        lib.axon_stop_nrt_profile.restype = ctypes.c_int64

        @contextlib.contextmanager
        def _hook(output_dir, device_ids):
            import jax
            jax.devices()
            if device_ids:
                ids = (ctypes.c_int64 * len(device_ids))(*device_ids)
                rc = lib.axon_start_nrt_profile(ids, len(device_ids))
            else:
                rc = lib.axon_start_nrt_profile(None, 0)
            if rc != 0:
                raise RuntimeError(f"axon_start_nrt_profile rc={rc}")
            try:
                yield
            finally:
                n = lib.axon_stop_nrt_profile(str(output_dir).encode())
                if n < 0:
                    raise RuntimeError(f"axon_stop_nrt_profile rc={n}")
                print(f"profile: {n} file(s) written to {output_dir}")

        return _hook

    def get_axon_ntff_profile_hook():
        if state["hook"] is None:
            try:
                state["hook"] = _build_hook()
            except Exception:
                state["hook"] = None
        return state["hook"]

    mod = types.ModuleType("antenv.axon_hooks")
    mod.get_axon_ntff_profile_hook = get_axon_ntff_profile_hook
    mod.set_axon_ntff_profile_hook = lambda h: state.update(hook=h)
    sys.modules["antenv.axon_hooks"] = mod


_install_ntff_hook_shim()


P = 128          # SBUF partitions
DIM = 1024       # model dim
KD = DIM // P    # 8 contraction tiles
ROWS_PER_CORE = 2048
SLABS_PER_CORE = 8
SLAB = 256       # rows per slab
N_CORES = 8
BF16 = mybir.dt.bfloat16
F32 = mybir.dt.float32

N_WARMUP_MM = 192

_CACHE = {}


def _build_graph():
    nc = bacc.Bacc("TRN2", target_bir_lowering=False, debug=False,
                   num_devices=N_CORES)

    # x and weights arrive host-pre-arranged in the exact SBUF layout so
    # every DMA descriptor is an 8-16KB contiguous run on both sides
    # (per-partition run length sets descriptor size; 1-2KB runs cap the
    # effective HBM read at ~195GB/s).
    xt_d = nc.dram_tensor("xt", [4, P, KD, 512], BF16, kind="ExternalInput")
    w_d = {
        name: nc.dram_tensor(name, [P, KD, DIM], BF16, kind="ExternalInput")
        for name in ("wq", "wk", "wv", "wp")
    }
    bqc_d = nc.dram_tensor("bqc", [P, DIM], F32, kind="ExternalInput")
    bkc_d = nc.dram_tensor("bkc", [P, DIM], F32, kind="ExternalInput")
    bpc_d = nc.dram_tensor("bpc", [P, DIM], F32, kind="ExternalInput")
    bvh_d = nc.dram_tensor("bvh", [64, P], BF16, kind="ExternalInput")
    ident_d = nc.dram_tensor("ident64", [64, 64], BF16, kind="ExternalInput")
    out_d = nc.dram_tensor("out", [ROWS_PER_CORE, DIM], BF16, kind="ExternalOutput")
    warm_d = nc.dram_tensor("warm", [64, 64], F32, kind="ExternalOutput")

    with tile.TileContext(nc) as tc:
        with (
            tc.tile_pool(name="wpool", bufs=1) as wpool,
            tc.tile_pool(name="xpool", bufs=1) as xpool,
            tc.tile_pool(name="bias", bufs=1) as bias_pool,
            tc.tile_pool(name="qk", bufs=2) as qk_pool,
            tc.tile_pool(name="vvt", bufs=2) as vvt_pool,
            tc.tile_pool(name="vt", bufs=1) as vt_pool,
            tc.tile_pool(name="ovt", bufs=2) as ovt_pool,
            tc.tile_pool(name="ysb", bufs=2) as y_pool,
            tc.tile_pool(name="soft", bufs=2) as soft_pool,
            tc.tile_pool(name="ps_qk", bufs=2, space="PSUM") as ps_qk_pool,
            tc.tile_pool(name="ps_v", bufs=2, space="PSUM") as ps_v_pool,
            tc.tile_pool(name="ps_y", bufs=2, space="PSUM") as ps_y_pool,
            tc.tile_pool(name="ps_swo", bufs=2, space="PSUM") as ps_swo_pool,
        ):
            # ---- resident tensors -------------------------------------------
            ident = bias_pool.tile([64, 64], BF16, tag="ident")
            nc.sync.dma_start(ident[:], ident_d[:])

            # PE warmup: dense tiny matmul chain so the HAM un-throttles
            # during the initial weight/activation DMA.  Consumed via the
            # "warm" output so it isn't dead code.
            ps_spam = ps_swo_pool.tile([64, 64], F32, tag="swo")
            for i in range(N_WARMUP_MM):
                nc.tensor.matmul(ps_spam[:], ident[:], ident[:],
                                 start=(i == 0), stop=(i == N_WARMUP_MM - 1))
            warm_sb = bias_pool.tile([64, 64], F32, tag="warm_sb")
            nc.vector.tensor_copy(warm_sb[:, 0:32], ps_spam[:, 0:32])

            bq_bc = bias_pool.tile([P, DIM], F32, tag="bqc")
            bk_bc = bias_pool.tile([P, DIM], F32, tag="bkc")
            bp_bc = bias_pool.tile([P, DIM], F32, tag="bpc")
            bvh_sb = bias_pool.tile([64, P], BF16, tag="bvh")
            nc.sync.dma_start(bq_bc[:], bqc_d[:])
            nc.sync.dma_start(bk_bc[:], bkc_d[:])
            nc.sync.dma_start(bp_bc[:], bpc_d[:])
            nc.sync.dma_start(bvh_sb[:], bvh_d[:])

            # xt pair-major so per-pair loads are contiguous on both sides
            xt_sb = xpool.tile([P, 4, KD, 512], BF16, tag="xt")
            w_sb = {}
            for name in ("wq", "wk", "wv", "wp"):
                w_sb[name] = wpool.tile([P, KD, DIM], BF16, tag=f"w_{name}",
                                        name=f"w_{name}")
            # DMA order tuned for startup (V of pair 0 runs first): xt
            # pair-0, wv, then wq/wk, then wp + xt rest.  Single large calls
            # (~1us SWDGE first-byte each) with full-run descriptors.
            nc.sync.dma_start(xt_sb[:, 0, :, :], xt_d[0])
            nc.sync.dma_start(w_sb["wv"][:], w_d["wv"][:])
            nc.sync.dma_start(w_sb["wq"][:], w_d["wq"][:])
            nc.sync.dma_start(w_sb["wk"][:], w_d["wk"][:])
            nc.sync.dma_start(w_sb["wp"][:], w_d["wp"][:])
            for q in range(1, 4):
                nc.sync.dma_start(xt_sb[:, q, :, :], xt_d[q])

            # ---- per slab-pair pipeline -------------------------------------
            # Emission (priority) order per pair:
            #   V(pair), part1(even), part1(odd), part2(even), part2(odd)
            # part1 = QK + S + softmax issue, part2 = WT/corr/OT/Y.  The PE
            # transpose in part2(s) sits AFTER the sibling slab's QK+S in the
            # PE stream, so each softmax's ~2.5us DVE/ACT latency hides under
            # the sibling's matmuls -- including for the final pair.

            def emit_v(pair):
                p0 = pair * 2 * SLAB
                # V^T for both slabs, evacuated into the head-interleaved
                # layout vvt[e, r, t2] (t = 16r + t2 flat, so OT's stationary
                # slices are single-free-dim contiguous).  The stride-16
                # scatter costs ~2.4us per [64,512] op on EVERY engine
                # (~5.6ns/elem): one fast contiguous CAST frees the PSUM slot
                # in ~0.7us, then ACT/GpSimd scatter from SBUF with ~10us of
                # deadline slack.  No bias here: the V-bias is folded into
                # the OT evac via the rank-1 correction C = P @ bv_head.
                vvt = vvt_pool.tile([64, 2 * SLAB, 16], BF16, tag="vvt")
                vt_tmp = vt_pool.tile([P, KD, 512], BF16, tag="vt_tmp")
                for jt in range(KD):
                    ps = ps_v_pool.tile([P, 512], F32, tag="ps_v")
                    for kd in range(KD):
                        nc.tensor.matmul(
                            ps[:],
                            w_sb["wv"][:, kd, jt * P:(jt + 1) * P],
                            xt_sb[:, pair, kd, :],
                            start=(kd == 0),
                            stop=(kd == KD - 1),
                        )
                    # V^T partition j = 128*jt + 64*par + e -> t2 = 2*jt+par,
                    # dst partition e, free col 16*r + t2.
                    nc.vector.tensor_copy(vt_tmp[:, jt, :], ps[:])
                    nc.scalar.copy(vvt[:, :, 2 * jt], vt_tmp[0:64, jt, :])
                    nc.gpsimd.tensor_copy(vvt[:, :, 2 * jt + 1],
                                          vt_tmp[64:128, jt, :])
                return vvt

            def emit_part1(s, paired):
                c0 = s * SLAB
                # Q, K natural layout (rows on partitions).  paired: jc0/jc1
                # chains interleaved so consecutive MMs share the xt
                # stationary.  Slab 0 runs unpaired so each chain can start
                # as soon as its 512-column weight half has landed.
                q_nat = qk_pool.tile([P, 2, DIM], BF16, tag="q_nat")
                k_nat = qk_pool.tile([P, 2, DIM], BF16, tag="k_nat")
                if not paired:
                    # jc-outer single chains: each starts as soon as its
                    # 512-column weight half has landed (startup slab only).
                    for jc in range(2):
                        for dst_t, wname, bias_bc in (
                            (q_nat, "wq", bq_bc),
                            (k_nat, "wk", bk_bc),
                        ):
                            for rt in range(2):
                                ps_a = ps_qk_pool.tile([P, 512], F32,
                                                       tag="ps_qk")
                                for kd in range(KD):
                                    o = (s % 2) * 256 + rt * P
                                    lhs = xt_sb[:, s // 2, kd, o: o + P]
                                    nc.tensor.matmul(
                                        ps_a[:], lhs,
                                        w_sb[wname][:, kd,
                                                    jc * 512:(jc + 1) * 512],
                                        start=(kd == 0), stop=(kd == KD - 1))
                                nc.vector.tensor_add(
                                    dst_t[:, rt, jc * 512:(jc + 1) * 512],
                                    ps_a[:],
                                    bias_bc[:, jc * 512:(jc + 1) * 512])
                for rt in range(2 if paired else 0):
                    for dst_t, wname, bias_bc in (
                        (q_nat, "wq", bq_bc),
                        (k_nat, "wk", bk_bc),
                    ):
                        if paired:
                            ps_a = ps_qk_pool.tile([P, 512], F32, tag="ps_qk")
                            ps_b = ps_qk_pool.tile([P, 512], F32, tag="ps_qk")
                            for kd in range(KD):
                                o = (s % 2) * 256 + rt * P
                                lhs = xt_sb[:, s // 2, kd, o: o + P]
                                nc.tensor.matmul(
                                    ps_a[:], lhs, w_sb[wname][:, kd, 0:512],
                                    start=(kd == 0), stop=(kd == KD - 1))
                                nc.tensor.matmul(
                                    ps_b[:], lhs, w_sb[wname][:, kd, 512:1024],
                                    start=(kd == 0), stop=(kd == KD - 1))
                            nc.vector.tensor_add(
                                dst_t[:, rt, 0:512], ps_a[:],
                                bias_bc[:, 0:512])
                            nc.vector.tensor_add(
                                dst_t[:, rt, 512:1024], ps_b[:],
                                bias_bc[:, 512:1024])
                        else:
                            pass  # unpaired slabs emit jc-outer below

                # S as 16 [128,128] MMs: each computes a 2x2 block of t2-pair
                # products; only the two diagonal 64x64 blocks are S
                # contributions (off-diagonals are discarded).  Halves the
                # S instruction count vs 32 single-t2 MMs.
                ps_s = ps_swo_pool.tile([P, P], F32, tag="swo")
                n_acc = 0
                for rt in range(2):
                    for tp in range(8):
                        nc.tensor.matmul(
                            ps_s[:],
                            q_nat[:, rt, tp * 128:(tp + 1) * 128],
                            k_nat[:, rt, tp * 128:(tp + 1) * 128],
                            start=(n_acc == 0),
                            stop=(n_acc == 15),
                        )
                        n_acc += 1
                d1_sb = soft_pool.tile([64, 64], F32, tag="d1_sb")
                nc.vector.tensor_copy(d1_sb[:], ps_s[64:128, 64:128])
                s_sb = soft_pool.tile([64, 64], F32, tag="s_sb")
                nc.vector.tensor_add(s_sb[:], ps_s[0:64, 0:64], d1_sb[:])

                # softmax over the free dim (DVE/ACT, overlaps PE)
                negmax = soft_pool.tile([64, 1], F32, tag="negmax")
                nc.vector.reduce_max(negmax[:], s_sb[:],
                                     axis=mybir.AxisListType.X, negate=True)
                p_sb = soft_pool.tile([64, 64], F32, tag="p_sb")
                rsum = soft_pool.tile([64, 1], F32, tag="rsum")
                nc.scalar.activation(p_sb[:], s_sb[:],
                                     mybir.ActivationFunctionType.Exp,
                                     bias=negmax[:], accum_out=rsum[:])
                rinv = soft_pool.tile([64, 1], F32, tag="rinv")
                nc.vector.reciprocal(rinv[:], rsum[:])
                w_soft = soft_pool.tile([64, 64], BF16, tag="w_soft")
                nc.vector.tensor_scalar_mul(w_soft[:], p_sb[:], rinv[:])
                return w_soft

            def emit_part2(s, half, vvt, w_soft, tail=False):
                # WT = W^T via PE transpose
                ps_wt = ps_swo_pool.tile([64, 64], BF16, tag="swo")
                nc.tensor.transpose(ps_wt[:], w_soft[:], ident[:])
                wt_sb = soft_pool.tile([64, 64], BF16, tag="wt_sb")
                nc.vector.tensor_copy(wt_sb[:], ps_wt[:])

                # V-bias correction: corr[16k+t2, d] = sum_e bvh[e,t2] *
                # P[d,e] -- one tiny MM; added to every O^T chunk (whose
                # partition p has t2 = p%16) during the ovt evac.
                corr_ps = ps_swo_pool.tile([P, 64], F32, tag="swo")
                nc.tensor.matmul(corr_ps[:], bvh_sb[:], wt_sb[:],
                                 start=True, stop=True)
                corr4 = soft_pool.tile([P, 4, 64], F32, tag="corr4")
                for t3 in range(4):
                    nc.vector.tensor_copy(corr4[:, t3, :], corr_ps[:])

                # O^T chunks; 4 chunks (t3=0..3) of one ct share a PSUM
                # tile [128, 4, 64], single evac interleaves into ovt.
                # lhsT = contiguous vvt slice (flat head-t columns).
                # In the wind-down (tail) there is no other PE work to cover
                # the pso evac cadence; borrow the idle ps_v banks for a
                # 4-deep rotation.
                ovt = ovt_pool.tile([P, KD, SLAB], BF16, tag="ovt")
                for ct in range(KD):
                    if tail and ct % 2 == 1:
                        pso = ps_v_pool.tile([P, 4, 64], F32, tag="ps_v")
                    else:
                        pso = ps_swo_pool.tile([P, 4, 64], F32, tag="swo")
                    for t3 in range(4):
                        c = 8 * t3 + ct
                        # chunk c: t in [128c, 128c+128) -> r in [8c,8c+8)
                        # contiguous [64, 8, 16] -> opts to [64, 128]
                        lhs = vvt[:, half * SLAB + c * 8:
                                  half * SLAB + (c + 1) * 8, :]
                        nc.tensor.matmul(
                            pso[:, t3, :],
                            lhs,
                            wt_sb[:],
                            start=True, stop=True,
                        )
                    nc.vector.tensor_add(
                        ovt[:, ct, :].rearrange("p (d four) -> p d four",
                                                four=4),
                        pso[:].rearrange("p t3 d -> p d t3"),
                        corr4[:].rearrange("p t3 d -> p d t3"),
                    )

                # Y = OvT^T @ Wp + bp; jc0/jc1 interleaved for stationary
                # reuse -> DMA out
                y_sb = y_pool.tile([P, 2, DIM], BF16, tag="y_sb")
                for rt in range(2):
                    ps_a = ps_y_pool.tile([P, 512], F32, tag="ps_y")
                    ps_b = ps_y_pool.tile([P, 512], F32, tag="ps_y")
                    for ct in range(KD):
                        lhs = ovt[:, ct, rt * P:(rt + 1) * P]
                        nc.tensor.matmul(
                            ps_a[:], lhs, w_sb["wp"][:, ct, 0:512],
                            start=(ct == 0), stop=(ct == KD - 1))
                        nc.tensor.matmul(
                            ps_b[:], lhs, w_sb["wp"][:, ct, 512:1024],
                            start=(ct == 0), stop=(ct == KD - 1))
                    nc.vector.tensor_add(
                        y_sb[:, rt, 0:512], ps_a[:], bp_bc[:, 0:512])
                    nc.vector.tensor_add(
                        y_sb[:, rt, 512:1024], ps_b[:], bp_bc[:, 512:1024])

                out_dst = out_d[s * SLAB:(s + 1) * SLAB, :] \
                    .rearrange("(rt p) c -> p rt c", p=P)
                nc.sync.dma_start(out_dst[:], y_sb[:])

            # V(p+1) is emitted mid-pair so its matmuls are available (and
            # ahead in static PE order) to fill the odd slab's softmax-chain
            # latency -- the sim underestimates that chain under SBUF
            # contention, so give it real filler.
            def emit_tail_spam(n, sl):
                # HAM keep-warm filler during the wind-down: the last pair
                # has no V/QK work left to cover softmax latency, and a few
                # sub-us PE gaps re-throttle the clock to 1.2GHz for the
                # final OT/Y matmuls.  ps_qk is idle by then.  Each chain
                # writes a distinct warm_sb slice so none is dead code.
                ps_f = ps_qk_pool.tile([64, 64], F32, tag="ps_qk")
                for i in range(n):
                    nc.tensor.matmul(ps_f[:], ident[:], ident[:],
                                     start=(i == 0), stop=(i == n - 1))
                nc.vector.tensor_copy(warm_sb[:, sl], ps_f[:, sl])

            vvt_cur = emit_v(0)
            last = SLABS_PER_CORE // 2 - 1
            for pair in range(SLABS_PER_CORE // 2):
                w0 = emit_part1(2 * pair, paired=True)
                w1 = emit_part1(2 * pair + 1, paired=True)
                if pair == last:
                    emit_tail_spam(16, slice(32, 48))
                emit_part2(2 * pair, 0, vvt_cur, w0, tail=(pair == last))
                vvt_next = None
                if pair < last:
                    vvt_next = emit_v(pair + 1)
                else:
                    emit_tail_spam(40, slice(48, 64))
                emit_part2(2 * pair + 1, 1, vvt_cur, w1, tail=(pair == last))
                vvt_cur = vvt_next
            nc.sync.dma_start(warm_d[:], warm_sb[:])

    nc.compile()
    return nc


def _prep_inputs(x, Wq, bq, Wk, bk, Wv, bv, Wp, bp):
    """Host-side shard prep. Returns in_maps list for 8 cores."""
    bf16 = ml_dtypes.bfloat16
    xf = np.ascontiguousarray(np.asarray(x, dtype=np.float32).reshape(-1, DIM))
    scale = np.float32(1.0 / np.sqrt(64.0))

    def warr(w):
        # [DIM, DIM] -> [P, KD, DIM]: partition-major, contiguous 16KB/part
        return np.ascontiguousarray(
            np.asarray(w, dtype=np.float32).astype(bf16)
            .reshape(KD, P, DIM).transpose(1, 0, 2))

    wq_b = warr(np.asarray(Wq) * scale)
    wk_b = warr(Wk)
    wv_b = warr(Wv)
    wp_b = warr(Wp)

    bqc = np.ascontiguousarray(np.broadcast_to(
        (np.asarray(bq) * scale).astype(np.float32), (P, DIM)))
    bkc = np.ascontiguousarray(np.broadcast_to(
        np.asarray(bk, dtype=np.float32), (P, DIM)))
    bpc = np.ascontiguousarray(np.broadcast_to(
        np.asarray(bp, dtype=np.float32), (P, DIM)))
    # bvh[e, 16k + t2] = bv[64*t2 + e]: lhsT of the per-slab V-bias
    # correction MM (out partition p=16k+t2 gets C[d, t2]).
    bvf = np.asarray(bv, dtype=np.float32).reshape(16, 64)  # [t2, e]
    bvh = np.ascontiguousarray(
        np.tile(bvf.T, (1, 8)).astype(bf16))                # [64, 128]
    ident = np.eye(64, dtype=bf16)

    shared = {
        "wq": wq_b, "wk": wk_b, "wv": wv_b, "wp": wp_b,
        "bqc": bqc, "bkc": bkc, "bpc": bpc, "bvh": bvh,
        "ident64": ident,
    }
    in_maps = []
    for c in range(N_CORES):
        xs = xf[c * ROWS_PER_CORE:(c + 1) * ROWS_PER_CORE]  # [2048, 1024]
        xsT = xs.T.astype(bf16)                             # [1024, 2048]
        # [4, P, KD, 512]: xt4[q, p, kd, r] = x^T[kd*128+p, 512q+r]
        xt4 = np.ascontiguousarray(
            xsT.reshape(KD, P, 4, 512).transpose(2, 1, 0, 3))
        in_maps.append({"xt": xt4, **shared})
    return in_maps


def kernel(x, Wq, bq, Wk, bk, Wv, bv, Wp, bp):
    if "nc" not in _CACHE:
        _CACHE["nc"] = _build_graph()
    nc = _CACHE["nc"]

    in_maps = _prep_inputs(x, Wq, bq, Wk, bk, Wv, bv, Wp, bp)
    trace = bool(int(os.environ.get("ATHENA_TRACE", "0")))
    res = run_bass_kernel_spmd(nc, in_maps, core_ids=list(range(N_CORES)),
                               trace=trace)
    _CACHE["last_exec_time_ns"] = res.exec_time_ns

    out = np.concatenate([res.results[c]["out"] for c in range(N_CORES)], axis=0)
    return np.ascontiguousarray(out.reshape(np.asarray(x).shape)
                                .astype(np.float32))


# revision 52
# speedup vs baseline: 1.2263x; 1.0109x over previous
"""Trainium2 Bass kernel for nn_Attention_1580547974448.

Math insight: the reference uses raw .reshape (not a head-split transpose) on
[B,T,H*HD] -> [B,H,T,HD].  With B=4, T=4096, DIM=1024, H=16, HD=64 this makes
each "head" a contiguous 256-row slab of the flattened [B*T, DIM] = [16384,1024]
input: for slab s (rows 256s..256s+255),
    Q = (x_s @ Wq + bq)            viewed row-major as [4096, 64]
    S = Q^T K / sqrt(64)           [64, 64]
    P = softmax(S, axis=-1)
    O = P @ V^T                    [64, 4096], row-major == [256, 1024]
    y_s = O_v @ Wp + bp
i.e. the whole computation is block-diagonal over 64 independent slabs.
We shard 8 slabs (2048 rows) per NeuronCore -> pure data parallel, no
collectives.  Compute dtype bf16 (fp32 PSUM accumulation).

Optimizations vs the 335us baseline (trace-driven; best ~297us,
~360us when the chip is P0 power-throttled to 2.0GHz):
  - Baseline's serializer was a GpSimd head-interleave of V^T (5.4us/copy,
    173us total): OT waited on it every slab, PE idled 46us and HAM
    re-throttled the clock (cold MMs 512ns vs warm 216ns).  The t2-to-free
    interleave is a stride-16 2B scatter costing ~2.4us per [64,512] op on
    EVERY engine (~5.6ns/elem, SBUF write RMW); it cannot be avoided (the
    matmul stationary AP must be single-free-dim) but it CAN be pipelined:
    one fast contiguous DVE CAST frees the V PSUM slot in ~0.7us, then ACT
    (even t2) and GpSimd (odd t2) scatter from SBUF with ~10us of slack.
  - V-bias folded into the OT evac as a rank-1 correction C = P @ bv_head
    (one [64x128x64] MM off WT per slab, added during the ovt evac), so
    scatters are plain copies any engine can run.
  - Q/K and Y emit jc0/jc1 chains interleaved: consecutive MMs share the
    stationary operand; a same-stationary MM issues at the 216ns stream
    floor vs ~270ns with a weight swap.
  - Emission order = scheduler priority: per pair, part1 = QK+S+softmax
    issue for BOTH slabs, then part2 = WT/corr/OT/Y, with V(p+1) emitted
    mid-pair -- each softmax's ~2.5us latency hides under sibling matmuls.
  - S computed as 16 [128,contract]x[128] MMs (t2-pairs; diagonal 64x64
    blocks summed by one DVE add) instead of 32 single-t2 MMs.
  - HAM keep-warm: ~280 tiny identity matmuls cover the DMA-bound start
    (DMA queues only begin ~8.6us in), small spam chains + ps_v-borrowed
    OT PSUM rotation keep the clock warm through the wind-down.  All spam
    consumed via the "warm" output (anti-DCE).
  - DMA: x and weights host-pre-arranged into the exact SBUF layout so
    every descriptor is an 8-16KB contiguous run on BOTH sides (1-2KB
    runs cap HBM read at ~195GB/s; full runs saturate all 16 queues,
    input done ~45us); few large calls (1us SWDGE first-byte each);
    bf16 output.
  - PSUM: qk 2 + v 2 + y 2 + (S/WT/corr/OT shared tag) 2 = 8 banks.

Per-core dataflow (all layouts [partition, free]):
  xt       [128, 4q, 8kd, 512]  x^T pair-major, bf16 (host pre-arranged)
  per pair: vvt [64, 512r, 16t2] = head-layout V^T (flat col = head t)
  per slab: q_nat/k_nat [128, 2rt, 1024] (DVE bias evac); S psum [128,128];
            softmax on free dim; WT via PE transpose; O^T 4-chunk PSUMs with
            lhsT = contiguous vvt slices; ovt [128, 8ct, 256]; y = ovt^T @ Wp.
"""

import os
import sys

import numpy as np
import ml_dtypes

import concourse.bass as bass
import concourse.mybir as mybir
import concourse.tile as tile
from concourse import bacc
from concourse.bass_utils import run_bass_kernel_spmd


def _install_ntff_hook_shim():
    """concourse's trace path does `from antenv.axon_hooks import
    get_axon_ntff_profile_hook`; this container's antenv lacks that
    module.  Provide it: a ctypes hook on the axon PJRT .so when
    available (mirrors trn_agent_boot), else a None hook (concourse
    then skips tracing gracefully)."""
    try:
        import antenv.axon_hooks  # noqa: F401
        return
    except ImportError:
        pass
    import contextlib
    import ctypes
    import types

    state = {"hook": None}

    def _build_hook():
        so_path = "/opt/axon/libaxon_pjrt.so"
        if not os.path.exists(so_path):
            return None
        lib = ctypes.CDLL(so_path)
        if not hasattr(lib, "axon_start_nrt_profile"):
            return None
        lib.axon_start_nrt_profile.argtypes = [
            ctypes.POINTER(ctypes.c_int64), ctypes.c_size_t]
        lib.axon_start_nrt_profile.restype = ctypes.c_int64
        lib.axon_stop_nrt_profile.argtypes = [ctypes.c_char_p]
        lib.axon_stop_nrt_profile.restype = ctypes.c_int64

        @contextlib.contextmanager
        def _hook(output_dir, device_ids):
            import jax
            jax.devices()
            if device_ids:
                ids = (ctypes.c_int64 * len(device_ids))(*device_ids)
                rc = lib.axon_start_nrt_profile(ids, len(device_ids))
            else:
                rc = lib.axon_start_nrt_profile(None, 0)
            if rc != 0:
                raise RuntimeError(f"axon_start_nrt_profile rc={rc}")
            try:
                yield
            finally:
                n = lib.axon_stop_nrt_profile(str(output_dir).encode())
                if n < 0:
                    raise RuntimeError(f"axon_stop_nrt_profile rc={n}")
                print(f"profile: {n} file(s) written to {output_dir}")

        return _hook

    def get_axon_ntff_profile_hook():
        if state["hook"] is None:
            try:
                state["hook"] = _build_hook()
            except Exception:
                state["hook"] = None
        return state["hook"]

    mod = types.ModuleType("antenv.axon_hooks")
    mod.get_axon_ntff_profile_hook = get_axon_ntff_profile_hook
    mod.set_axon_ntff_profile_hook = lambda h: state.update(hook=h)
    sys.modules["antenv.axon_hooks"] = mod


_install_ntff_hook_shim()


P = 128          # SBUF partitions
DIM = 1024       # model dim
KD = DIM // P    # 8 contraction tiles
ROWS_PER_CORE = 2048
SLABS_PER_CORE = 8
SLAB = 256       # rows per slab
N_CORES = 8
BF16 = mybir.dt.bfloat16
F32 = mybir.dt.float32

N_WARMUP_MM = 192

_CACHE = {}


def _build_graph():
    nc = bacc.Bacc("TRN2", target_bir_lowering=False, debug=False,
                   num_devices=N_CORES)

    # x and weights arrive host-pre-arranged in the exact SBUF layout so
    # every DMA descriptor is an 8-16KB contiguous run on both sides
    # (per-partition run length sets descriptor size; 1-2KB runs cap the
    # effective HBM read at ~195GB/s).
    xt_d = nc.dram_tensor("xt", [4, P, KD, 512], BF16, kind="ExternalInput")
    w_d = {
        name: nc.dram_tensor(name, [P, KD, DIM], BF16, kind="ExternalInput")
        for name in ("wq", "wk", "wp")
    }
    # wv split in column halves (jt 0-3 / 4-7) so V chains start on 1MB
    w_d["wv"] = nc.dram_tensor("wv", [P, 2, KD, 512], BF16,
                               kind="ExternalInput")
    bqc_d = nc.dram_tensor("bqc", [P, DIM], F32, kind="ExternalInput")
    bkc_d = nc.dram_tensor("bkc", [P, DIM], F32, kind="ExternalInput")
    bpc_d = nc.dram_tensor("bpc", [P, DIM], F32, kind="ExternalInput")
    bvh_d = nc.dram_tensor("bvh", [64, P], BF16, kind="ExternalInput")
    ident_d = nc.dram_tensor("ident64", [64, 64], BF16, kind="ExternalInput")
    out_d = nc.dram_tensor("out", [ROWS_PER_CORE, DIM], BF16, kind="ExternalOutput")
    warm_d = nc.dram_tensor("warm", [64, 64], F32, kind="ExternalOutput")

    with tile.TileContext(nc) as tc:
        with (
            tc.tile_pool(name="wpool", bufs=1) as wpool,
            tc.tile_pool(name="xpool", bufs=1) as xpool,
            tc.tile_pool(name="bias", bufs=1) as bias_pool,
            tc.tile_pool(name="qk", bufs=2) as qk_pool,
            tc.tile_pool(name="vvt", bufs=2) as vvt_pool,
            tc.tile_pool(name="vt", bufs=1) as vt_pool,
            tc.tile_pool(name="ovt", bufs=2) as ovt_pool,
            tc.tile_pool(name="ysb", bufs=2) as y_pool,
            tc.tile_pool(name="soft", bufs=2) as soft_pool,
            tc.tile_pool(name="ps_qk", bufs=2, space="PSUM") as ps_qk_pool,
            tc.tile_pool(name="ps_v", bufs=2, space="PSUM") as ps_v_pool,
            tc.tile_pool(name="ps_y", bufs=2, space="PSUM") as ps_y_pool,
            tc.tile_pool(name="ps_swo", bufs=2, space="PSUM") as ps_swo_pool,
        ):
            # ---- resident tensors -------------------------------------------
            ident = bias_pool.tile([64, 64], BF16, tag="ident")
            nc.sync.dma_start(ident[:], ident_d[:])

            # PE warmup: dense tiny matmul chain so the HAM un-throttles
            # during the initial weight/activation DMA.  Consumed via the
            # "warm" output so it isn't dead code.
            ps_spam = ps_swo_pool.tile([64, 64], F32, tag="swo")
            for i in range(N_WARMUP_MM):
                nc.tensor.matmul(ps_spam[:], ident[:], ident[:],
                                 start=(i == 0), stop=(i == N_WARMUP_MM - 1))
            warm_sb = bias_pool.tile([64, 64], F32, tag="warm_sb")
            nc.vector.tensor_copy(warm_sb[:, 0:32], ps_spam[:, 0:32])

            bq_bc = bias_pool.tile([P, DIM], F32, tag="bqc")
            bk_bc = bias_pool.tile([P, DIM], F32, tag="bkc")
            bp_bc = bias_pool.tile([P, DIM], F32, tag="bpc")
            bvh_sb = bias_pool.tile([64, P], BF16, tag="bvh")
            nc.sync.dma_start(bq_bc[:], bqc_d[:])
            nc.sync.dma_start(bk_bc[:], bkc_d[:])
            nc.sync.dma_start(bp_bc[:], bpc_d[:])
            nc.sync.dma_start(bvh_sb[:], bvh_d[:])

            # xt pair-major so per-pair loads are contiguous on both sides
            xt_sb = xpool.tile([P, 4, KD, 512], BF16, tag="xt")
            w_sb = {}
            for name in ("wq", "wk", "wp"):
                w_sb[name] = wpool.tile([P, KD, DIM], BF16, tag=f"w_{name}",
                                        name=f"w_{name}")
            w_sb["wv"] = wpool.tile([P, 2, KD, 512], BF16, tag="w_wv",
                                    name="w_wv")
            # DMA order tuned for startup (V of pair 0 runs first): xt
            # pair-0, wv, then wq/wk, then wp + xt rest.  Single large calls
            # (~1us SWDGE first-byte each) with full-run descriptors.
            nc.sync.dma_start(xt_sb[:, 0, :, :], xt_d[0])
            nc.sync.dma_start(w_sb["wv"][:, 0], w_d["wv"][:, 0])
            nc.sync.dma_start(w_sb["wv"][:, 1], w_d["wv"][:, 1])
            nc.sync.dma_start(w_sb["wq"][:], w_d["wq"][:])
            nc.sync.dma_start(w_sb["wk"][:], w_d["wk"][:])
            nc.sync.dma_start(w_sb["wp"][:], w_d["wp"][:])
            for q in range(1, 4):
                nc.sync.dma_start(xt_sb[:, q, :, :], xt_d[q])

            # ---- per slab-pair pipeline -------------------------------------
            # Emission (priority) order per pair:
            #   V(pair), part1(even), part1(odd), part2(even), part2(odd)
            # part1 = QK + S + softmax issue, part2 = WT/corr/OT/Y.  The PE
            # transpose in part2(s) sits AFTER the sibling slab's QK+S in the
            # PE stream, so each softmax's ~2.5us DVE/ACT latency hides under
            # the sibling's matmuls -- including for the final pair.

            def emit_v(pair):
                p0 = pair * 2 * SLAB
                # V^T for both slabs, evacuated into the head-interleaved
                # layout vvt[e, r, t2] (t = 16r + t2 flat, so OT's stationary
                # slices are single-free-dim contiguous).  The stride-16
                # scatter costs ~2.4us per [64,512] op on EVERY engine
                # (~5.6ns/elem): one fast contiguous CAST frees the PSUM slot
                # in ~0.7us, then ACT/GpSimd scatter from SBUF with ~10us of
                # deadline slack.  No bias here: the V-bias is folded into
                # the OT evac via the rank-1 correction C = P @ bv_head.
                vvt = vvt_pool.tile([64, 2 * SLAB, 16], BF16, tag="vvt")
                vt_tmp = vt_pool.tile([P, KD, 512], BF16, tag="vt_tmp")
                for jt in range(KD):
                    ps = ps_v_pool.tile([P, 512], F32, tag="ps_v")
                    for kd in range(KD):
                        nc.tensor.matmul(
                            ps[:],
                            w_sb["wv"][:, jt // 4, kd,
                                        (jt % 4) * P:(jt % 4 + 1) * P],
                            xt_sb[:, pair, kd, :],
                            start=(kd == 0),
                            stop=(kd == KD - 1),
                        )
                    # V^T partition j = 128*jt + 64*par + e -> t2 = 2*jt+par,
                    # dst partition e, free col 16*r + t2.
                    nc.vector.tensor_copy(vt_tmp[:, jt, :], ps[:])
                    nc.scalar.copy(vvt[:, :, 2 * jt], vt_tmp[0:64, jt, :])
                    nc.gpsimd.tensor_copy(vvt[:, :, 2 * jt + 1],
                                          vt_tmp[64:128, jt, :])
                return vvt

            def emit_part1(s, paired):
                c0 = s * SLAB
                # Q, K natural layout (rows on partitions).  paired: jc0/jc1
                # chains interleaved so consecutive MMs share the xt
                # stationary.  Slab 0 runs unpaired so each chain can start
                # as soon as its 512-column weight half has landed.
                q_nat = qk_pool.tile([P, 2, DIM], BF16, tag="q_nat")
                k_nat = qk_pool.tile([P, 2, DIM], BF16, tag="k_nat")
                if not paired:
                    # jc-outer single chains: each starts as soon as its
                    # 512-column weight half has landed (startup slab only).
                    for jc in range(2):
                        for dst_t, wname, bias_bc in (
                            (q_nat, "wq", bq_bc),
                            (k_nat, "wk", bk_bc),
                        ):
                            for rt in range(2):
                                ps_a = ps_qk_pool.tile([P, 512], F32,
                                                       tag="ps_qk")
                                for kd in range(KD):
                                    o = (s % 2) * 256 + rt * P
                                    lhs = xt_sb[:, s // 2, kd, o: o + P]
                                    nc.tensor.matmul(
                                        ps_a[:], lhs,
                                        w_sb[wname][:, kd,
                                                    jc * 512:(jc + 1) * 512],
                                        start=(kd == 0), stop=(kd == KD - 1))
                                nc.vector.tensor_add(
                                    dst_t[:, rt, jc * 512:(jc + 1) * 512],
                                    ps_a[:],
                                    bias_bc[:, jc * 512:(jc + 1) * 512])
                for rt in range(2 if paired else 0):
                    for dst_t, wname, bias_bc in (
                        (q_nat, "wq", bq_bc),
                        (k_nat, "wk", bk_bc),
                    ):
                        if paired:
                            ps_a = ps_qk_pool.tile([P, 512], F32, tag="ps_qk")
                            ps_b = ps_qk_pool.tile([P, 512], F32, tag="ps_qk")
                            for kd in range(KD):
                                o = (s % 2) * 256 + rt * P
                                lhs = xt_sb[:, s // 2, kd, o: o + P]
                                nc.tensor.matmul(
                                    ps_a[:], lhs, w_sb[wname][:, kd, 0:512],
                                    start=(kd == 0), stop=(kd == KD - 1))
                                nc.tensor.matmul(
                                    ps_b[:], lhs, w_sb[wname][:, kd, 512:1024],
                                    start=(kd == 0), stop=(kd == KD - 1))
                            nc.vector.tensor_add(
                                dst_t[:, rt, 0:512], ps_a[:],
                                bias_bc[:, 0:512])
                            nc.vector.tensor_add(
                                dst_t[:, rt, 512:1024], ps_b[:],
                                bias_bc[:, 512:1024])
                        else:
                            pass  # unpaired slabs emit jc-outer below

                # S as 16 [128,128] MMs: each computes a 2x2 block of t2-pair
                # products; only the two diagonal 64x64 blocks are S
                # contributions (off-diagonals are discarded).  Halves the
                # S instruction count vs 32 single-t2 MMs.
                ps_s = ps_swo_pool.tile([P, P], F32, tag="swo")
                n_acc = 0
                for rt in range(2):
                    for tp in range(8):
                        nc.tensor.matmul(
                            ps_s[:],
                            q_nat[:, rt, tp * 128:(tp + 1) * 128],
                            k_nat[:, rt, tp * 128:(tp + 1) * 128],
                            start=(n_acc == 0),
                            stop=(n_acc == 15),
                        )
                        n_acc += 1
                d1_sb = soft_pool.tile([64, 64], F32, tag="d1_sb")
                nc.vector.tensor_copy(d1_sb[:], ps_s[64:128, 64:128])
                s_sb = soft_pool.tile([64, 64], F32, tag="s_sb")
                nc.vector.tensor_add(s_sb[:], ps_s[0:64, 0:64], d1_sb[:])

                # softmax over the free dim (DVE/ACT, overlaps PE)
                negmax = soft_pool.tile([64, 1], F32, tag="negmax")
                nc.vector.reduce_max(negmax[:], s_sb[:],
                                     axis=mybir.AxisListType.X, negate=True)
                p_sb = soft_pool.tile([64, 64], F32, tag="p_sb")
                rsum = soft_pool.tile([64, 1], F32, tag="rsum")
                nc.scalar.activation(p_sb[:], s_sb[:],
                                     mybir.ActivationFunctionType.Exp,
                                     bias=negmax[:], accum_out=rsum[:])
                rinv = soft_pool.tile([64, 1], F32, tag="rinv")
                nc.vector.reciprocal(rinv[:], rsum[:])
                w_soft = soft_pool.tile([64, 64], BF16, tag="w_soft")
                nc.vector.tensor_scalar_mul(w_soft[:], p_sb[:], rinv[:])
                return w_soft

            def emit_part2(s, half, vvt, w_soft, tail=False):
                # WT = W^T via PE transpose
                ps_wt = ps_swo_pool.tile([64, 64], BF16, tag="swo")
                nc.tensor.transpose(ps_wt[:], w_soft[:], ident[:])
                wt_sb = soft_pool.tile([64, 64], BF16, tag="wt_sb")
                nc.vector.tensor_copy(wt_sb[:], ps_wt[:])

                # V-bias correction: corr[16k+t2, d] = sum_e bvh[e,t2] *
                # P[d,e] -- one tiny MM; added to every O^T chunk (whose
                # partition p has t2 = p%16) during the ovt evac.
                corr_ps = ps_swo_pool.tile([P, 64], F32, tag="swo")
                nc.tensor.matmul(corr_ps[:], bvh_sb[:], wt_sb[:],
                                 start=True, stop=True)
                corr4 = soft_pool.tile([P, 4, 64], F32, tag="corr4")
                for t3 in range(4):
                    nc.vector.tensor_copy(corr4[:, t3, :], corr_ps[:])

                # O^T chunks; 4 chunks (t3=0..3) of one ct share a PSUM
                # tile [128, 4, 64], single evac interleaves into ovt.
                # lhsT = contiguous vvt slice (flat head-t columns).
                # In the wind-down (tail) there is no other PE work to cover
                # the pso evac cadence; borrow the idle ps_v banks for a
                # 4-deep rotation.
                ovt = ovt_pool.tile([P, KD, SLAB], BF16, tag="ovt")
                for ct in range(KD):
                    if tail and ct % 2 == 1:
                        pso = ps_v_pool.tile([P, 4, 64], F32, tag="ps_v")
                    else:
                        pso = ps_swo_pool.tile([P, 4, 64], F32, tag="swo")
                    for t3 in range(4):
                        c = 8 * t3 + ct
                        # chunk c: t in [128c, 128c+128) -> r in [8c,8c+8)
                        # contiguous [64, 8, 16] -> opts to [64, 128]
                        lhs = vvt[:, half * SLAB + c * 8:
                                  half * SLAB + (c + 1) * 8, :]
                        nc.tensor.matmul(
                            pso[:, t3, :],
                            lhs,
                            wt_sb[:],
                            start=True, stop=True,
                        )
                    nc.vector.tensor_add(
                        ovt[:, ct, :].rearrange("p (d four) -> p d four",
                                                four=4),
                        pso[:].rearrange("p t3 d -> p d t3"),
                        corr4[:].rearrange("p t3 d -> p d t3"),
                    )

                # Y = OvT^T @ Wp + bp; jc0/jc1 interleaved for stationary
                # reuse -> DMA out
                y_sb = y_pool.tile([P, 2, DIM], BF16, tag="y_sb")
                for rt in range(2):
                    ps_a = ps_y_pool.tile([P, 512], F32, tag="ps_y")
                    ps_b = ps_y_pool.tile([P, 512], F32, tag="ps_y")
                    for ct in range(KD):
                        lhs = ovt[:, ct, rt * P:(rt + 1) * P]
                        nc.tensor.matmul(
                            ps_a[:], lhs, w_sb["wp"][:, ct, 0:512],
                            start=(ct == 0), stop=(ct == KD - 1))
                        nc.tensor.matmul(
                            ps_b[:], lhs, w_sb["wp"][:, ct, 512:1024],
                            start=(ct == 0), stop=(ct == KD - 1))
                    nc.vector.tensor_add(
                        y_sb[:, rt, 0:512], ps_a[:], bp_bc[:, 0:512])
                    nc.vector.tensor_add(
                        y_sb[:, rt, 512:1024], ps_b[:], bp_bc[:, 512:1024])

                out_dst = out_d[s * SLAB:(s + 1) * SLAB, :] \
                    .rearrange("(rt p) c -> p rt c", p=P)
                nc.sync.dma_start(out_dst[:], y_sb[:])

            # V(p+1) is emitted mid-pair so its matmuls are available (and
            # ahead in static PE order) to fill the odd slab's softmax-chain
            # latency -- the sim underestimates that chain under SBUF
            # contention, so give it real filler.
            def emit_tail_spam(n, sl):
                # HAM keep-warm filler during the wind-down: the last pair
                # has no V/QK work left to cover softmax latency, and a few
                # sub-us PE gaps re-throttle the clock to 1.2GHz for the
                # final OT/Y matmuls.  ps_qk is idle by then.  Each chain
                # writes a distinct warm_sb slice so none is dead code.
                ps_f = ps_qk_pool.tile([64, 64], F32, tag="ps_qk")
                for i in range(n):
                    nc.tensor.matmul(ps_f[:], ident[:], ident[:],
                                     start=(i == 0), stop=(i == n - 1))
                nc.vector.tensor_copy(warm_sb[:, sl], ps_f[:, sl])

            vvt_cur = emit_v(0)
            last = SLABS_PER_CORE // 2 - 1
            for pair in range(SLABS_PER_CORE // 2):
                w0 = emit_part1(2 * pair, paired=True)
                w1 = emit_part1(2 * pair + 1, paired=True)
                if pair == last:
                    emit_tail_spam(16, slice(32, 48))
                emit_part2(2 * pair, 0, vvt_cur, w0, tail=(pair == last))
                vvt_next = None
                if pair < last:
                    vvt_next = emit_v(pair + 1)
                else:
                    emit_tail_spam(40, slice(48, 64))
                emit_part2(2 * pair + 1, 1, vvt_cur, w1, tail=(pair == last))
                vvt_cur = vvt_next
            nc.sync.dma_start(warm_d[:], warm_sb[:])

    nc.compile()
    return nc


def _prep_inputs(x, Wq, bq, Wk, bk, Wv, bv, Wp, bp):
    """Host-side shard prep. Returns in_maps list for 8 cores."""
    bf16 = ml_dtypes.bfloat16
    xf = np.ascontiguousarray(np.asarray(x, dtype=np.float32).reshape(-1, DIM))
    scale = np.float32(1.0 / np.sqrt(64.0))

    def warr(w):
        # [DIM, DIM] -> [P, KD, DIM]: partition-major, contiguous 16KB/part
        return np.ascontiguousarray(
            np.asarray(w, dtype=np.float32).astype(bf16)
            .reshape(KD, P, DIM).transpose(1, 0, 2))

    wq_b = warr(np.asarray(Wq) * scale)
    wk_b = warr(Wk)
    # wv additionally split in column halves: [P, 2, KD, 512]
    wv_b = np.ascontiguousarray(
        warr(Wv).reshape(P, KD, 2, 512).transpose(0, 2, 1, 3))
    wp_b = warr(Wp)

    bqc = np.ascontiguousarray(np.broadcast_to(
        (np.asarray(bq) * scale).astype(np.float32), (P, DIM)))
    bkc = np.ascontiguousarray(np.broadcast_to(
        np.asarray(bk, dtype=np.float32), (P, DIM)))
    bpc = np.ascontiguousarray(np.broadcast_to(
        np.asarray(bp, dtype=np.float32), (P, DIM)))
    # bvh[e, 16k + t2] = bv[64*t2 + e]: lhsT of the per-slab V-bias
    # correction MM (out partition p=16k+t2 gets C[d, t2]).
    bvf = np.asarray(bv, dtype=np.float32).reshape(16, 64)  # [t2, e]
    bvh = np.ascontiguousarray(
        np.tile(bvf.T, (1, 8)).astype(bf16))                # [64, 128]
    ident = np.eye(64, dtype=bf16)

    shared = {
        "wq": wq_b, "wk": wk_b, "wv": wv_b, "wp": wp_b,
        "bqc": bqc, "bkc": bkc, "bpc": bpc, "bvh": bvh,
        "ident64": ident,
    }
    in_maps = []
    for c in range(N_CORES):
        xs = xf[c * ROWS_PER_CORE:(c + 1) * ROWS_PER_CORE]  # [2048, 1024]
        xsT = xs.T.astype(bf16)                             # [1024, 2048]
        # [4, P, KD, 512]: xt4[q, p, kd, r] = x^T[kd*128+p, 512q+r]
        xt4 = np.ascontiguousarray(
            xsT.reshape(KD, P, 4, 512).transpose(2, 1, 0, 3))
        in_maps.append({"xt": xt4, **shared})
    return in_maps


def kernel(x, Wq, bq, Wk, bk, Wv, bv, Wp, bp):
    if "nc" not in _CACHE:
        _CACHE["nc"] = _build_graph()
    nc = _CACHE["nc"]

    in_maps = _prep_inputs(x, Wq, bq, Wk, bk, Wv, bv, Wp, bp)
    trace = bool(int(os.environ.get("ATHENA_TRACE", "0")))
    res = run_bass_kernel_spmd(nc, in_maps, core_ids=list(range(N_CORES)),
                               trace=trace)
    _CACHE["last_exec_time_ns"] = res.exec_time_ns

    out = np.concatenate([res.results[c]["out"] for c in range(N_CORES)], axis=0)
    return np.ascontiguousarray(out.reshape(np.asarray(x).shape)
                                .astype(np.float32))
